# revision 1
# baseline (speedup 1.0000x reference)
"""2-layer GCN (PyG GCNConv semantics) on 8 Trainium2 NeuronCores.

Strategy (self-contained; shapes hardcoded for the nn_GCNEncoder problem):
  - Nodes are relabeled (degree-balanced) and partitioned across 8 cores
    (12544 padded nodes each, 98 blocks of 128).
  - Layer math is refactored so every edge message is a 64-wide row gather:
      L1: table1 = dinv * (x @ W1)            (dense matmul per shard + AllGather)
          out1   = relu(dinv * segsum(table1[src]) + b1)
      L2: table2 = dinv * out1                (W2 applied AFTER aggregation)
          out    = (dinv * segsum(table2[src])) @ W2 + b2
  - Edge aggregation per 128-node dst block: gather 128-edge chunks with
    dma_gather (int16 idx, 4 segments, 4 SWDGE queues), build one-hot
    dst-selection matrices on the DVE (is_equal vs iota), and scatter-add via
    PSUM-accumulated TensorE matmuls.
"""
import math
import numpy as np
from contextlib import ExitStack

N_REAL = 100000
N_PAD = 100352            # 8 * 98 * 128
NCORES = 8
NSHARD = N_PAD // NCORES  # 12544
NBLOCKS = NSHARD // 128   # 98
P = 128
SEG = 25088               # int16 segment size (4 segments)
NSEG = 4
F1 = 64                   # hidden width (W1 out)
F2 = 32                   # output width (W2 out)
IN_C = 128
C_BATCH = 16              # chunks per DVE one-hot build
SENTINEL = 500.0


def _bf16(a):
    import jax.numpy as jnp
    return np.asarray(jnp.asarray(a, dtype=jnp.bfloat16))


def _balanced_perm(deg):
    """Assign nodes to 784 blocks of 128 balancing per-block degree sums.
    Returns perm: orig_id -> new_id (new_id = block*128 + slot)."""
    import heapq
    nblocks_g = (N_PAD // P)  # 784
    order = np.argsort(-deg, kind="stable")
    # LPT greedy: biggest-degree node to the currently-lightest block with a
    # free slot (blocks capped at 128 nodes)
    blocks = np.empty(N_REAL, np.int64)
    heap = [(0, b) for b in range(nblocks_g)]
    heapq.heapify(heap)
    fill = np.zeros(nblocks_g, np.int64)
    deg_sorted = deg[order]
    for i in range(N_REAL):
        load, b = heapq.heappop(heap)
        blocks[order[i]] = b
        fill[b] += 1
        if fill[b] < P:
            heapq.heappush(heap, (load + int(deg_sorted[i]), b))
    # slot within block: stable order
    perm = np.empty(N_PAD, np.int64)
    slot_counter = np.zeros(nblocks_g, np.int64)
    # vectorized slot assignment: sort by block
    o2 = np.argsort(blocks, kind="stable")
    counts = np.bincount(blocks, minlength=nblocks_g)
    starts = np.concatenate([[0], np.cumsum(counts)[:-1]])
    slots = np.arange(N_REAL) - starts[blocks[o2]]
    new_ids = blocks[o2] * P + slots
    perm_real = np.empty(N_REAL, np.int64)
    perm_real[o2] = new_ids
    # pad nodes fill remaining slots
    used = np.zeros(N_PAD, bool)
    used[perm_real] = True
    free_ids = np.flatnonzero(~used)
    perm = np.concatenate([perm_real, free_ids])
    return perm  # length N_PAD; first N_REAL entries map real nodes


def _build_schedule(new_src, new_dst):
    """new_src/new_dst: int64 arrays over all edges (new ids).
    Returns (call_list, idx16 arrays per core, dst arrays per core, stats).
    call_list: [(block, seg, k_chunks), ...] in schedule order (uniform across cores).
    """
    core = new_dst // NSHARD
    block_l = (new_dst % NSHARD) // P          # 0..97
    dst_l = new_dst % P
    seg = new_src // SEG
    src_l = new_src % SEG

    # counts per (core, block, seg)
    key = (core * NBLOCKS + block_l) * NSEG + seg
    counts = np.bincount(key, minlength=NCORES * NBLOCKS * NSEG).reshape(
        NCORES, NBLOCKS, NSEG)
    k_bs = np.maximum(1, np.ceil(counts.max(axis=0) / P).astype(np.int64))  # [NBLOCKS, NSEG]

    nchunk = int(k_bs.sum())
    nidx = nchunk * P

    # order edges by (core, block, seg)
    eorder = np.lexsort((seg, block_l, core))
    s_src = src_l[eorder]
    s_dst = dst_l[eorder]
    s_key = key[eorder]

    # slot base for each (core, block, seg) cell in the per-core flat idx array
    cell_off = np.zeros((NBLOCKS, NSEG), np.int64)
    running = 0
    call_list = []
    for b in range(NBLOCKS):
        for s in range(NSEG):
            cell_off[b, s] = running
            call_list.append((b, s, int(k_bs[b, s])))
            running += int(k_bs[b, s]) * P
    assert running == nidx

    # position of each edge within its cell
    cnt_flat = counts.reshape(-1)
    starts = np.concatenate([[0], np.cumsum(cnt_flat)[:-1]])
    within = np.arange(len(s_src)) - starts[s_key]
    cell_core = s_key // (NBLOCKS * NSEG)
    cell_bs = s_key % (NBLOCKS * NSEG)
    cell_b = cell_bs // NSEG
    cell_s = cell_bs % NSEG
    flat_pos = cell_off[cell_b, cell_s] + within   # position in core's [nidx] array

    idx_arr = np.zeros((NCORES, nidx), np.int32)  # pad rows gather row 0 of the segment (harmless; dst sentinel zeroes them)
    dst_arr = np.full((NCORES, nidx), int(SENTINEL), np.int32)
    idx_arr[cell_core, flat_pos] = s_src
    dst_arr[cell_core, flat_pos] = s_dst
    return call_list, idx_arr, dst_arr, nchunk, nidx, counts, k_bs


def _wrap_idx(idx_flat, call_list):
    """Per-call wrapped int16 layout: each call's n idxs -> [16, n/16] block,
    replicated to 128 partitions; concatenated along columns."""
    nidx = idx_flat.shape[0]
    cols = np.empty((16, nidx // 16), np.int16)
    off = 0
    coloff = 0
    for (b, s, k) in call_list:
        n = k * P
        blockv = idx_flat[off:off + n].astype(np.int16).reshape(n // 16, 16).T
        cols[:, coloff:coloff + n // 16] = blockv
        off += n
        coloff += n // 16
    return np.tile(cols, (8, 1))  # [128, nidx//16]


def _make_runner(nc, n_cores):
    """Build the bass2jax PJRT executable once; return a callable that runs it
    (optionally repeatedly, for timing)."""
    import time
    import jax
    from jax.sharding import Mesh, PartitionSpec
    from jax.experimental.shard_map import shard_map
    import concourse.mybir as mybir
    from concourse.bass2jax import (_bass_exec_p, install_neuronx_cc_hook,
                                    partition_id_tensor)

    install_neuronx_cc_hook()
    partition_name = nc.partition_id_tensor.name if nc.partition_id_tensor else None

    in_names, out_names, out_avals, zero_outs = [], [], [], []
    for alloc in nc.m.functions[0].allocations:
        if not isinstance(alloc, mybir.MemoryLocationSet):
            continue
        name = alloc.memorylocations[0].name
        if alloc.kind == "ExternalInput":
            if name != partition_name:
                in_names.append(name)
        elif alloc.kind == "ExternalOutput":
            out_names.append(name)
            shape = tuple(alloc.tensor_shape)
            dtype = mybir.dt.np(alloc.dtype)
            out_avals.append(jax.core.ShapedArray(shape, dtype))
            zero_outs.append(np.zeros(shape, dtype))
    n_params = len(in_names)
    n_outs = len(out_avals)
    all_in_names = list(in_names) + list(out_names)
    if partition_name is not None:
        all_in_names.append(partition_name)

    def _body(*args):
        operands = list(args)
        if partition_name is not None:
            operands.append(partition_id_tensor())
        outs = _bass_exec_p.bind(
            *operands,
            out_avals=tuple(out_avals),
            in_names=tuple(all_in_names),
            out_names=tuple(out_names),
            lowering_input_output_aliases=(),
            sim_require_finite=False,
            sim_require_nnan=False,
            nc=nc,
        )
        return tuple(outs)

    devices = jax.devices()[:n_cores]
    mesh = Mesh(np.asarray(devices), ("core",))
    in_specs = (PartitionSpec("core"),) * (n_params + n_outs)
    out_specs = (PartitionSpec("core"),) * len(out_names)
    sharded = jax.jit(
        shard_map(_body, mesh=mesh, in_specs=in_specs, out_specs=out_specs,
                  check_rep=False),
        keep_unused=True,
    )

    def run(in_maps, n_iters=0):
        per_core = [[np.asarray(m[name]) for name in in_names] for m in in_maps]
        concat_in = [
            np.concatenate([per_core[c][i] for c in range(n_cores)], axis=0)
            for i in range(n_params)
        ]
        concat_zeros = [
            np.zeros((n_cores * z.shape[0], *z.shape[1:]), z.dtype) for z in zero_outs
        ]
        args = [jax.device_put(a) for a in concat_in + concat_zeros]
        out = sharded(*args)
        jax.block_until_ready(out)
        times = []
        for _ in range(n_iters):
            t0 = time.perf_counter()
            out = sharded(*args)
            jax.block_until_ready(out)
            times.append(time.perf_counter() - t0)
        results = [
            {name: np.asarray(out[i]).reshape(n_cores, *out_avals[i].shape)[c]
             for i, name in enumerate(out_names)}
            for c in range(n_cores)
        ]
        return results, times

    return run


def _compile_and_make_runner(call_list, nchunk, nidx, with_b1, with_b2):
    import os
    import concourse.bass as bass
    import concourse.bacc as bacc
    import concourse.mybir as mybir
    import concourse.tile as tile
    from concourse.bass import exact_div

    STAGE = int(os.environ.get("KSTAGE", "4"))
    dt = mybir.dt

    def dma_gather(gp, out_ap, in_ap, idxs_ap, num_idxs, elem_size, elem_step, q):
        stride_bytes_256 = exact_div(elem_step * dt.size(in_ap.dtype), 256)
        _in_ap = gp.lower_ap_dma(in_ap, for_custom_bir_dma=True)
        return gp.add_instruction(
            mybir.InstDMAGatherAnt(
                name=gp.bass.get_next_instruction_name(),
                ins=[*_in_ap, gp.lower_ap(idxs_ap),
                     gp.lower_val_access(gp.to_reg(num_idxs))],
                outs=[gp.lower_ap(out_ap)],
                transpose=False, num_idxs=num_idxs, elem_size=elem_size,
                stride_bytes_256=stride_bytes_256, gen_mode=0,
                single_packet=False, queue_num=q))

    nc = bacc.Bacc("TRN2", target_bir_lowering=False, debug=False,
                   num_devices=NCORES, num_swdge_queues=4)

    xT = nc.dram_tensor("xT", [P, NSHARD], dt.bfloat16, kind="ExternalInput").ap()
    w1 = nc.dram_tensor("w1", [P, F1], dt.bfloat16, kind="ExternalInput").ap()
    w2 = nc.dram_tensor("w2", [F1, F2], dt.bfloat16, kind="ExternalInput").ap()
    dinvb = nc.dram_tensor("dinvb", [P, NBLOCKS], dt.float32, kind="ExternalInput").ap()
    idx16 = nc.dram_tensor("idx16", [P, nidx // 16], dt.int16, kind="ExternalInput").ap()
    dstloc = nc.dram_tensor("dstloc", [P, nchunk], dt.bfloat16, kind="ExternalInput").ap()
    iota = nc.dram_tensor("iota", [P, P], dt.bfloat16, kind="ExternalInput").ap()
    ident = nc.dram_tensor("ident", [P, P], dt.bfloat16, kind="ExternalInput").ap()
    if with_b1:
        b1bc = nc.dram_tensor("b1bc", [P, F1], dt.float32, kind="ExternalInput").ap()
    if with_b2:
        b2bc = nc.dram_tensor("b2bc", [P, F2], dt.float32, kind="ExternalInput").ap()
    out = nc.dram_tensor("out", [NSHARD, F2], dt.float32, kind="ExternalOutput").ap()

    g1_shard = nc.dram_tensor("g1_shard", [NSHARD, 128], dt.bfloat16).ap()
    g1_full = nc.dram_tensor("g1_full", [N_PAD, 128], dt.bfloat16, addr_space="Shared").ap()
    g2_shard = nc.dram_tensor("g2_shard", [NSHARD, 128], dt.bfloat16).ap()
    g2_full = nc.dram_tensor("g2_full", [N_PAD, 128], dt.bfloat16, addr_space="Shared").ap()

    KMAX = max(k for (_, _, k) in call_list)

    with tile.TileContext(nc) as tc, ExitStack() as ctx:
        sb = ctx.enter_context(tc.tile_pool(name="sb", bufs=1))
        sbg = ctx.enter_context(tc.tile_pool(name="sbg", bufs=24))
        sbs = ctx.enter_context(tc.tile_pool(name="sbs", bufs=8))
        sbb = ctx.enter_context(tc.tile_pool(name="sbb", bufs=8))
        ps_agg = ctx.enter_context(tc.tile_pool(name="psagg", bufs=4, space="PSUM"))
        ps_t = ctx.enter_context(tc.tile_pool(name="pst", bufs=2, space="PSUM"))
        ps_s = ctx.enter_context(tc.tile_pool(name="pss", bufs=2, space="PSUM"))

        # ---- resident tiles ----
        xT_sb = sb.tile([P, NSHARD], dt.bfloat16)
        w1_sb = sb.tile([P, F1], dt.bfloat16)
        w2_sb = sb.tile([F1, F2], dt.bfloat16)
        dinv_sb = sb.tile([P, NBLOCKS], dt.float32)
        idx_sb = sb.tile([P, nidx // 16], dt.int16)
        dst_sb = sb.tile([P, nchunk], dt.bfloat16)
        iota_sb = sb.tile([P, P], dt.bfloat16)
        ident_sb = sb.tile([P, P], dt.bfloat16)
        nc.sync.dma_start(xT_sb[:], xT)
        nc.sync.dma_start(w1_sb[:], w1)
        nc.sync.dma_start(w2_sb[:], w2)
        nc.sync.dma_start(dinv_sb[:], dinvb)
        nc.sync.dma_start(idx_sb[:], idx16)
        nc.sync.dma_start(dst_sb[:], dstloc)
        nc.sync.dma_start(iota_sb[:], iota)
        nc.sync.dma_start(ident_sb[:], ident)
        if with_b1:
            b1_sb = sb.tile([P, F1], dt.float32)
            nc.sync.dma_start(b1_sb[:], b1bc)
        if with_b2:
            b2_sb = sb.tile([P, F2], dt.float32)
            nc.sync.dma_start(b2_sb[:], b2bc)

        g1_stage = sb.tile([P, NBLOCKS * F1], dt.bfloat16)
        g2_stage = sb.tile([P, NBLOCKS * F1], dt.bfloat16)
        out_stage = sb.tile([P, NBLOCKS * F2], dt.float32)

        # ---- phase 1: table1 = dinv * (x @ W1) (dinv folded into xT on host) ----
        for t in range(NBLOCKS):
            h_ps = ps_s.tile([P, F1], dt.float32, tag="smallps")
            nc.tensor.matmul(h_ps[:], lhsT=xT_sb[:, t * P:(t + 1) * P],
                             rhs=w1_sb[:], start=True, stop=True)
            nc.scalar.copy(g1_stage[:, t * F1:(t + 1) * F1], h_ps[:])
        nc.sync.dma_start(
            g1_shard.rearrange("(b p) f -> p b f", p=P)[:, :, :F1],
            g1_stage[:].rearrange("p (b f) -> p b f", f=F1))
        nc.gpsimd.collective_compute(
            "AllGather", mybir.AluOpType.bypass,
            replica_groups=[list(range(NCORES))],
            ins=[g1_shard], outs=[g1_full])

        # ---- edge phase helper ----
        def edge_phase(g_full_ap, stage_sb, post_block):
            # one-hot builds batched over chunks; gathers per (block, seg) call
            s2_tiles = {}

            def get_s2(ch):
                b0 = (ch // C_BATCH) * C_BATCH
                if b0 not in s2_tiles:
                    cb = min(C_BATCH, nchunk - b0)
                    s2 = sbs.tile([P, C_BATCH * P], dt.bfloat16, tag="s2")
                    nc.vector.tensor_tensor(
                        out=s2[:].rearrange("p (c j) -> p c j", c=C_BATCH)[:, :cb, :],
                        in0=iota_sb[:, None, :].to_broadcast((P, cb, P)),
                        in1=dst_sb[:, b0:b0 + cb, None].to_broadcast((P, cb, P)),
                        op=mybir.AluOpType.is_equal)
                    s2_tiles.clear()
                    s2_tiles[b0] = s2
                return s2_tiles[b0], ch - b0

            ch = 0
            idxcol = 0
            qn = 0
            agg_of_block = {}
            for ci, (b, s, k) in enumerate(call_list):
                gb = sbg.tile([P, KMAX * F1], dt.bfloat16, tag="gbuf")
                n = k * P
                dma_gather(
                    nc.gpsimd,
                    gb[:, :k * F1].rearrange("p (g f) -> p g f", f=F1),
                    g_full_ap[s * SEG:(s + 1) * SEG, :F1],
                    idx_sb[:, idxcol:idxcol + n // 16],
                    num_idxs=n, elem_size=F1, elem_step=128, q=qn)
                qn = (qn + 1) % 4
                idxcol += n // 16
                if s == 0:
                    # self-loop contribution: agg = I.T @ table_rows(block)
                    agg_of_block[b] = ps_agg.tile([P, F1], dt.float32, tag="agg", name="aggps")
                    nc.tensor.matmul(
                        agg_of_block[b][:], lhsT=ident_sb[:],
                        rhs=stage_sb[:, b * F1:(b + 1) * F1],
                        start=True, stop=False)
                for c in range(k):
                    sl = agg_of_block[b][:]
                    s2, coff = get_s2(ch)
                    is_last = (s == NSEG - 1 and c == k - 1)
                    nc.tensor.matmul(
                        sl, lhsT=s2[:, coff * P:(coff + 1) * P],
                        rhs=gb[:, c * F1:(c + 1) * F1],
                        start=False, stop=is_last)
                    ch += 1
                if s == NSEG - 1:
                    post_block(b, agg_of_block.pop(b)[:])

        # ---- L1 post: out1 = relu(dinv*agg [+ b1]); table2 = dinv*out1 ----
        def post1(b, agg_sl):
            dv = dinv_sb[:, b:b + 1]
            if with_b1:
                t = sbb.tile([P, F1], dt.float32, tag="pb1")
                nc.vector.tensor_scalar(t[:], agg_sl, dv, None,
                                        mybir.AluOpType.mult)
                nc.vector.tensor_tensor(t[:], t[:], b1_sb[:], mybir.AluOpType.add)
                h1 = sbb.tile([P, F1], dt.bfloat16, tag="ph1")
                nc.scalar.activation(h1[:], t[:], mybir.ActivationFunctionType.Relu)
            else:
                h1 = sbb.tile([P, F1], dt.bfloat16, tag="ph1")
                nc.scalar.activation(h1[:], agg_sl, mybir.ActivationFunctionType.Relu,
                                     bias=0.0, scale=dv)
            nc.vector.tensor_scalar(g2_stage[:, b * F1:(b + 1) * F1], h1[:], dv, None,
                                    mybir.AluOpType.mult)

        if STAGE >= 2:
            edge_phase(g1_full, g1_stage, post1)
        else:
            nc.vector.tensor_copy(g2_stage[:, :], g1_stage[:, :])
        nc.sync.dma_start(
            g2_shard.rearrange("(b p) f -> p b f", p=P)[:, :, :F1],
            g2_stage[:].rearrange("p (b f) -> p b f", f=F1))
        if STAGE >= 3:
            nc.gpsimd.collective_compute(
                "AllGather", mybir.AluOpType.bypass,
                replica_groups=[list(range(NCORES))],
                ins=[g2_shard], outs=[g2_full])

        # ---- L2 post: out = (dinv*agg2) @ W2 [+ b2] ----
        def post2(b, agg_sl):
            dv = dinv_sb[:, b:b + 1]
            a2 = sbb.tile([P, F1], dt.bfloat16, tag="pa2")
            nc.scalar.activation(a2[:], agg_sl, mybir.ActivationFunctionType.Copy,
                                 bias=0.0, scale=dv)
            a2t_ps = ps_t.tile([F1, P], dt.bfloat16, tag="tps")
            nc.tensor.transpose(a2t_ps[:], a2[:], ident_sb[:])
            a2t = sbb.tile([F1, P], dt.bfloat16, tag="pa2t")
            nc.vector.tensor_copy(a2t[:], a2t_ps[:])
            o_ps = ps_s.tile([P, F1], dt.float32, tag="smallps")
            nc.tensor.matmul(o_ps[:, :F2], lhsT=a2t[:], rhs=w2_sb[:],
                             start=True, stop=True)
            osl = out_stage[:, b * F2:(b + 1) * F2]
            if with_b2:
                nc.vector.tensor_tensor(osl, o_ps[:, :F2], b2_sb[:],
                                        mybir.AluOpType.add)
            else:
                nc.vector.tensor_copy(osl, o_ps[:, :F2])

        if STAGE >= 4:
            edge_phase(g2_full, g2_stage, post2)
        else:
            for b in range(NBLOCKS):
                nc.vector.tensor_copy(out_stage[:, b * F2:(b + 1) * F2],
                                      g2_stage[:, b * F1:b * F1 + F2])
        nc.sync.dma_start(
            out.rearrange("(b p) f -> p b f", p=P),
            out_stage[:].rearrange("p (b f) -> p b f", f=F2))

    nc.compile()
    return _make_runner(nc, NCORES)


_CACHE = {}


def kernel(x, edge_index, W1, b1, W2, b2):
    x = np.asarray(x, np.float32)
    edge_index = np.asarray(edge_index)
    W1 = np.asarray(W1, np.float32)
    b1 = np.asarray(b1, np.float32)
    W2 = np.asarray(W2, np.float32)
    b2 = np.asarray(b2, np.float32)

    src = edge_index[0].astype(np.int64)
    dst = edge_index[1].astype(np.int64)
    # degree includes self-loops (GCNConv adds one per node)
    deg = (np.bincount(dst, minlength=N_REAL) + 1).astype(np.float64)
    dinv = (1.0 / np.sqrt(deg)).astype(np.float32)

    perm = _balanced_perm(np.bincount(dst, minlength=N_REAL))  # orig -> new
    # self-loops are handled separately on-device (identity matmul against the
    # block's own table rows); schedule only the real edges
    new_src = perm[src]
    new_dst = perm[dst]

    dinv_new = np.zeros(N_PAD, np.float32)
    dinv_new[perm[:N_REAL]] = dinv

    call_list, idx_arr, dst_arr, nchunk, nidx, counts, k_bs = _build_schedule(
        new_src, new_dst)

    with_b1 = bool(np.any(b1))
    with_b2 = bool(np.any(b2))

    ckey = (nchunk, nidx, with_b1, with_b2, tuple(k for (_, _, k) in call_list))
    if ckey not in _CACHE:
        _CACHE[ckey] = _compile_and_make_runner(
            call_list, nchunk, nidx, with_b1, with_b2)
    run = _CACHE[ckey]
    globals()['_last_runner'] = run

    # host-side inputs per core
    x_new = np.zeros((N_PAD, IN_C), np.float32)
    x_new[perm[:N_REAL]] = x
    xs = dinv_new[:, None] * x_new            # fold dinv into x
    iota_np = np.broadcast_to(np.arange(P, dtype=np.float32), (P, P)).copy()
    ident_np = np.eye(P, dtype=np.float32)
    w1_bf = _bf16(W1)
    w2_bf = _bf16(W2)
    iota_bf = _bf16(iota_np)
    ident_bf = _bf16(ident_np)

    in_maps = []
    for c in range(NCORES):
        lo, hi = c * NSHARD, (c + 1) * NSHARD
        m = {
            "xT": _bf16(xs[lo:hi].T.copy()),
            "w1": w1_bf,
            "w2": w2_bf,
            "dinvb": dinv_new[lo:hi].reshape(NBLOCKS, P).T.copy(),
            "idx16": _wrap_idx(idx_arr[c], call_list),
            "dstloc": _bf16(dst_arr[c].astype(np.float32).reshape(nchunk, P).T.copy()),
            "iota": iota_bf,
            "ident": ident_bf,
        }
        if with_b1:
            m["b1bc"] = np.broadcast_to(b1, (P, F1)).copy()
        if with_b2:
            m["b2bc"] = np.broadcast_to(b2, (P, F2)).copy()
        in_maps.append(m)

    globals()['_last_in_maps'] = in_maps
    results, _times = run(in_maps, n_iters=0)
    out_new = np.concatenate([results[c]["out"] for c in range(NCORES)], axis=0)
    return out_new[perm[:N_REAL]].astype(np.float32)



# revision 10
# speedup vs baseline: 2.9763x; 2.9763x over previous
"""2-layer GCN (PyG GCNConv semantics) on 8 Trainium2 NeuronCores.

Strategy (self-contained; shapes hardcoded for the nn_GCNEncoder problem):
  - Nodes are relabeled (degree-balanced) and partitioned across 8 cores
    (12544 padded nodes each, 98 blocks of 128). Within a block, slot
    p = (j%2)*64 + j//2 so even/odd "pair" halves occupy partitions 0-63 /
    64-127, letting message tables pack two 64-wide node rows per 256-byte
    DRAM row (halves AllGather volume; gather rows stay 256B-stride).
  - Layer math refactored so every edge message is a 64-wide row gather:
      L1: table1 = dinv * (x @ W1)         (per-shard matmul + AllGather)
          table2 = dinv^2 * relu(segsum(table1[src]))     (b1 == 0 fast path)
      L2: out    = (dinv * segsum(table2[src])) @ W2 + b2
  - Edge aggregation per 128-node dst block: SWDGE gathers merged per
    (14-block group x (segment,parity)) to amortize the ~1us per-instruction
    GpSimd descriptor-generation cost; one-hot dst-selection matrices built
    on the DVE (is_equal vs iota); scatter-add via PSUM-accumulated TensorE
    matmuls; per-block epilogues on the Activation engine.
"""
import math
import numpy as np
from contextlib import ExitStack

N_REAL = 100000
N_PAD = 100352            # 8 * 98 * 128
NCORES = 8
NSHARD = N_PAD // NCORES  # 12544
NBLOCKS = NSHARD // 128   # 98
P = 128
HALF = 64
NPAIR_SH = NSHARD // 2    # 6272 packed pair-rows per core
NPAIR = N_PAD // 2        # 50176
SEGP = NPAIR // 2         # 25088 pair rows per int16 segment (2 segments)
NGRP = 4                  # (segment, parity) source groups
F1 = 64                   # hidden width (W1 out)
F2 = 32                   # output width (W2 out)
IN_C = 128
G_BLK = 14                # dst blocks per merged gather call (98 = 7*14)
NBG = NBLOCKS // G_BLK    # 7
C_BATCH = 32              # chunks per DVE one-hot build
SENTINEL = 500.0


def _bf16(a):
    import jax.numpy as jnp
    return np.asarray(jnp.asarray(a, dtype=jnp.bfloat16))


def _balanced_perm(deg):
    """Assign nodes to 784 blocks of 128 balancing per-block degree sums.
    Returns perm: orig_id -> new_id (new_id = block*128 + slot)."""
    import heapq
    nblocks_g = (N_PAD // P)  # 784
    order = np.argsort(-deg, kind="stable")
    blocks = np.empty(N_REAL, np.int64)
    heap = [(0, b) for b in range(nblocks_g)]
    heapq.heapify(heap)
    fill = np.zeros(nblocks_g, np.int64)
    deg_sorted = deg[order]
    for i in range(N_REAL):
        load, b = heapq.heappop(heap)
        blocks[order[i]] = b
        fill[b] += 1
        if fill[b] < P:
            heapq.heappush(heap, (load + int(deg_sorted[i]), b))
    o2 = np.argsort(blocks, kind="stable")
    counts = np.bincount(blocks, minlength=nblocks_g)
    starts = np.concatenate([[0], np.cumsum(counts)[:-1]])
    slots = np.arange(N_REAL) - starts[blocks[o2]]
    new_ids = blocks[o2] * P + slots
    perm_real = np.empty(N_REAL, np.int64)
    perm_real[o2] = new_ids
    used = np.zeros(N_PAD, bool)
    used[perm_real] = True
    free_ids = np.flatnonzero(~used)
    perm = np.concatenate([perm_real, free_ids])
    return perm  # length N_PAD; first N_REAL entries map real nodes


def _build_schedule(new_src, new_dst):
    """new_src/new_dst: int64 arrays over all edges (new ids, slot-twisted).
    Returns (call_list, idx arrays per core, dst arrays per core, nchunk, nidx).
    call_list: [(g, [(b, k), ...], n_chunks, chunk0)] in schedule order
    (uniform across cores); one dma_gather per entry."""
    p_s = new_src % P
    two = p_s // HALF
    q = p_s % HALF
    pr = (new_src // NSHARD) * NPAIR_SH + ((new_src % NSHARD) // P) * HALF + q
    grp = (pr // SEGP) * 2 + two          # 0..3
    idxv = pr % SEGP                      # < 25088, int16-safe

    core = new_dst // NSHARD
    block = (new_dst % NSHARD) // P       # 0..97
    dslot = new_dst % P

    key = (core * NBLOCKS + block) * NGRP + grp
    counts = np.bincount(key, minlength=NCORES * NBLOCKS * NGRP).reshape(
        NCORES, NBLOCKS, NGRP)
    k_bs = np.maximum(1, np.ceil(counts.max(axis=0) / P).astype(np.int64))

    cell_off = np.zeros((NBLOCKS, NGRP), np.int64)
    call_list = []
    running = 0
    chunk_run = 0
    for bgi in range(NBG):
        for g in range(NGRP):
            cells = []
            nch = 0
            for b in range(bgi * G_BLK, (bgi + 1) * G_BLK):
                cell_off[b, g] = running + nch * P
                cells.append((b, int(k_bs[b, g])))
                nch += int(k_bs[b, g])
            call_list.append((g, cells, nch, chunk_run))
            running += nch * P
            chunk_run += nch
    nidx = running
    nchunk = chunk_run

    eorder = np.lexsort((grp, block, core))
    s_idx = idxv[eorder]
    s_dslot = dslot[eorder]
    s_key = key[eorder]
    cnt_flat = counts.reshape(-1)
    starts = np.concatenate([[0], np.cumsum(cnt_flat)[:-1]])
    within = np.arange(len(s_idx)) - starts[s_key]
    cell_core = s_key // (NBLOCKS * NGRP)
    cbg = s_key % (NBLOCKS * NGRP)
    flat_pos = cell_off[cbg // NGRP, cbg % NGRP] + within

    # pad slots gather row 0 of their segment (harmless; dst sentinel zeroes)
    idx_arr = np.zeros((NCORES, nidx), np.int32)
    dst_arr = np.full((NCORES, nidx), int(SENTINEL), np.int32)
    idx_arr[cell_core, flat_pos] = s_idx
    dst_arr[cell_core, flat_pos] = s_dslot

    # consumption order is block-major within each block group (a block's
    # PSUM accumulation chains across the group's 4 gather calls); dst
    # columns are permuted host-side to that order.
    cons_order = []
    for bgi in range(NBG):
        calls = call_list[bgi * NGRP:(bgi + 1) * NGRP]
        for b in range(bgi * G_BLK, (bgi + 1) * G_BLK):
            for (g, cells, nch, c0) in calls:
                coff = 0
                for (b2, k) in cells:
                    if b2 == b:
                        cons_order.extend(range(c0 + coff, c0 + coff + k))
                        break
                    coff += k
    cons_order = np.asarray(cons_order)
    assert len(cons_order) == nchunk
    dst_cons = dst_arr.reshape(NCORES, nchunk, P)[:, cons_order, :]
    return call_list, idx_arr, dst_cons, nchunk, nidx


def _wrap_idx(idx_flat, call_list):
    """Per-call wrapped int16 layout: each call's n idxs -> [16, n/16] block,
    replicated to 128 partitions; concatenated along columns."""
    nidx = idx_flat.shape[0]
    cols = np.empty((16, nidx // 16), np.int16)
    off = 0
    coloff = 0
    for (g, cells, nch, c0) in call_list:
        n = nch * P
        blockv = idx_flat[off:off + n].astype(np.int16).reshape(n // 16, 16).T
        cols[:, coloff:coloff + n // 16] = blockv
        off += n
        coloff += n // 16
    return np.tile(cols, (8, 1))  # [128, nidx//16]


def _make_runner(nc, n_cores):
    """Build the bass2jax PJRT executable once; return a callable that runs it
    (optionally repeatedly, for timing)."""
    import time
    import jax
    from jax.sharding import Mesh, PartitionSpec, NamedSharding
    from jax.experimental.shard_map import shard_map
    import concourse.mybir as mybir
    from concourse.bass2jax import (_bass_exec_p, install_neuronx_cc_hook,
                                    partition_id_tensor)

    install_neuronx_cc_hook()
    partition_name = nc.partition_id_tensor.name if nc.partition_id_tensor else None

    in_names, out_names, out_avals, zero_outs = [], [], [], []
    for alloc in nc.m.functions[0].allocations:
        if not isinstance(alloc, mybir.MemoryLocationSet):
            continue
        name = alloc.memorylocations[0].name
        if alloc.kind == "ExternalInput":
            if name != partition_name:
                in_names.append(name)
        elif alloc.kind == "ExternalOutput":
            out_names.append(name)
            shape = tuple(alloc.tensor_shape)
            dtype = mybir.dt.np(alloc.dtype)
            out_avals.append(jax.core.ShapedArray(shape, dtype))
            zero_outs.append(np.zeros(shape, dtype))
    n_params = len(in_names)
    n_outs = len(out_avals)
    all_in_names = list(in_names) + list(out_names)
    if partition_name is not None:
        all_in_names.append(partition_name)

    def _body(*args):
        operands = list(args)
        if partition_name is not None:
            operands.append(partition_id_tensor())
        outs = _bass_exec_p.bind(
            *operands,
            out_avals=tuple(out_avals),
            in_names=tuple(all_in_names),
            out_names=tuple(out_names),
            lowering_input_output_aliases=(),
            sim_require_finite=False,
            sim_require_nnan=False,
            nc=nc,
        )
        return tuple(outs)

    devices = jax.devices()[:n_cores]
    mesh = Mesh(np.asarray(devices), ("core",))
    in_specs = (PartitionSpec("core"),) * (n_params + n_outs)
    out_specs = (PartitionSpec("core"),) * len(out_names)
    sharded = jax.jit(
        shard_map(_body, mesh=mesh, in_specs=in_specs, out_specs=out_specs,
                  check_rep=False),
        keep_unused=True,
    )

    def run(in_maps, n_iters=0):
        shard = NamedSharding(mesh, PartitionSpec("core"))
        per_core = [[np.asarray(m[name]) for name in in_names] for m in in_maps]
        concat_in = [
            np.concatenate([per_core[c][i] for c in range(n_cores)], axis=0)
            for i in range(n_params)
        ]
        concat_zeros = [
            np.zeros((n_cores * z.shape[0], *z.shape[1:]), z.dtype) for z in zero_outs
        ]
        args = [jax.device_put(a, shard) for a in concat_in + concat_zeros]
        out = sharded(*args)
        jax.block_until_ready(out)
        times = []
        for _ in range(n_iters):
            t0 = time.perf_counter()
            out = sharded(*args)
            jax.block_until_ready(out)
            times.append(time.perf_counter() - t0)
        results = [
            {name: np.asarray(out[i]).reshape(n_cores, *out_avals[i].shape)[c]
             for i, name in enumerate(out_names)}
            for c in range(n_cores)
        ]
        return results, times

    return run


def _compile_and_make_runner(call_list, nchunk, nidx, with_b1, with_b2):
    import concourse.bass as bass
    import concourse.bacc as bacc
    import concourse.mybir as mybir
    import concourse.tile as tile
    from concourse.bass import exact_div

    dt = mybir.dt

    def dma_gather(gp, out_ap, in_ap, idxs_ap, num_idxs, elem_size, elem_step, q):
        stride_bytes_256 = exact_div(elem_step * dt.size(in_ap.dtype), 256)
        _in_ap = gp.lower_ap_dma(in_ap, for_custom_bir_dma=True)
        return gp.add_instruction(
            mybir.InstDMAGatherAnt(
                name=gp.bass.get_next_instruction_name(),
                ins=[*_in_ap, gp.lower_ap(idxs_ap),
                     gp.lower_val_access(gp.to_reg(num_idxs))],
                outs=[gp.lower_ap(out_ap)],
                transpose=False, num_idxs=num_idxs, elem_size=elem_size,
                stride_bytes_256=stride_bytes_256, gen_mode=0,
                single_packet=False, queue_num=q))

    nc = bacc.Bacc("TRN2", target_bir_lowering=False, debug=False,
                   num_devices=NCORES, num_swdge_queues=4)

    xT = nc.dram_tensor("xT", [P, NSHARD], dt.bfloat16, kind="ExternalInput").ap()
    w1 = nc.dram_tensor("w1", [P, F1], dt.bfloat16, kind="ExternalInput").ap()
    w2 = nc.dram_tensor("w2", [F1, F2], dt.bfloat16, kind="ExternalInput").ap()
    dinvb = nc.dram_tensor("dinvb", [P, NBLOCKS], dt.float32, kind="ExternalInput").ap()
    dinv2b = nc.dram_tensor("dinv2b", [P, NBLOCKS], dt.float32, kind="ExternalInput").ap()
    idx16 = nc.dram_tensor("idx16", [P, nidx // 16], dt.int16, kind="ExternalInput").ap()
    dstloc = nc.dram_tensor("dstloc", [P, nchunk], dt.bfloat16, kind="ExternalInput").ap()
    iota = nc.dram_tensor("iota", [P, P], dt.bfloat16, kind="ExternalInput").ap()
    ident = nc.dram_tensor("ident", [P, P], dt.bfloat16, kind="ExternalInput").ap()
    if with_b1:
        b1bc = nc.dram_tensor("b1bc", [P, F1], dt.float32, kind="ExternalInput").ap()
    if with_b2:
        b2bc = nc.dram_tensor("b2bc", [P, F2], dt.float32, kind="ExternalInput").ap()
    out = nc.dram_tensor("out", [NSHARD, F2], dt.float32, kind="ExternalOutput").ap()

    g1_shard = nc.dram_tensor("g1_shard", [NPAIR_SH, P], dt.bfloat16).ap()
    g1_full = nc.dram_tensor("g1_full", [NPAIR, P], dt.bfloat16, addr_space="Shared").ap()
    g2_shard = nc.dram_tensor("g2_shard", [NPAIR_SH, P], dt.bfloat16).ap()
    g2_full = nc.dram_tensor("g2_full", [NPAIR, P], dt.bfloat16, addr_space="Shared").ap()

    NCMAX = max(nch for (_, _, nch, _) in call_list)

    with tile.TileContext(nc) as tc, ExitStack() as ctx:
        sb = ctx.enter_context(tc.tile_pool(name="sb", bufs=1))
        sbx = ctx.enter_context(tc.tile_pool(name="sbx", bufs=4))
        sbg = ctx.enter_context(tc.tile_pool(name="sbg", bufs=2 * NGRP))
        sbs = ctx.enter_context(tc.tile_pool(name="sbs", bufs=3))
        sbb = ctx.enter_context(tc.tile_pool(name="sbb", bufs=8))
        ps_agg = ctx.enter_context(tc.tile_pool(name="psagg", bufs=4,
                                                space="PSUM"))
        ps_t = ctx.enter_context(tc.tile_pool(name="pst", bufs=2, space="PSUM"))
        ps_s = ctx.enter_context(tc.tile_pool(name="pss", bufs=2, space="PSUM"))

        # ---- resident tiles ----
        w1_sb = sb.tile([P, F1], dt.bfloat16)
        w2_sb = sb.tile([F1, F2], dt.bfloat16)
        dinv_sb = sb.tile([P, NBLOCKS], dt.float32)
        dinv2_sb = sb.tile([P, NBLOCKS], dt.float32)
        idx_sb = sb.tile([P, nidx // 16], dt.int16)
        dst_sb = sb.tile([P, nchunk], dt.bfloat16)
        iota_sb = sb.tile([P, P], dt.bfloat16)
        ident_sb = sb.tile([P, P], dt.bfloat16)
        nc.sync.dma_start(w1_sb[:], w1)
        nc.sync.dma_start(w2_sb[:], w2)
        nc.sync.dma_start(dinv_sb[:], dinvb)
        nc.sync.dma_start(dinv2_sb[:], dinv2b)
        nc.sync.dma_start(idx_sb[:], idx16)
        nc.sync.dma_start(dst_sb[:], dstloc)
        nc.sync.dma_start(iota_sb[:], iota)
        nc.sync.dma_start(ident_sb[:], ident)
        if with_b1:
            b1_sb = sb.tile([P, F1], dt.float32)
            nc.sync.dma_start(b1_sb[:], b1bc)
        if with_b2:
            b2_sb = sb.tile([P, F2], dt.float32)
            nc.sync.dma_start(b2_sb[:], b2bc)

        g1_stage = sb.tile([P, NBLOCKS * F1], dt.bfloat16)
        g2_stage = sb.tile([P, NBLOCKS * F1], dt.bfloat16)
        out_stage = sb.tile([P, NBLOCKS * F2], dt.float32)

        # ---- phase 1: table1 = dinv * (x @ W1) (dinv folded into xT on host;
        # xT streamed per block to keep SBUF for gather buffers) ----
        for t in range(NBLOCKS):
            xt_t = sbx.tile([P, P], dt.bfloat16, tag="xt")
            nc.sync.dma_start(xt_t[:], xT[:, t * P:(t + 1) * P])
            h_ps = ps_s.tile([P, F1], dt.float32, tag="smallps")
            nc.tensor.matmul(h_ps[:], lhsT=xt_t[:], rhs=w1_sb[:],
                             start=True, stop=True)
            nc.scalar.copy(g1_stage[:, t * F1:(t + 1) * F1], h_ps[:])

        def packed_write(g_shard_ap, stage_sb):
            # slot p = two*64 + q holds node (b, q*2+two); pair row b*64+q
            gv = g_shard_ap.rearrange("(b q) (two f) -> q two b f", q=HALF, two=2)
            nc.sync.dma_start(
                gv[:, 0],
                stage_sb[0:HALF, :].rearrange("q (b f) -> q b f", f=F1))
            nc.sync.dma_start(
                gv[:, 1],
                stage_sb[HALF:P, :].rearrange("q (b f) -> q b f", f=F1))

        packed_write(g1_shard, g1_stage[:])
        nc.gpsimd.collective_compute(
            "AllGather", mybir.AluOpType.bypass,
            replica_groups=[list(range(NCORES))],
            ins=[g1_shard], outs=[g1_full])

        # ---- edge phase helper ----
        # gathers merged per (block-group, source-group); consumption is
        # block-major: each block's PSUM accumulation chains across the
        # group's 4 gather buffers. dst_sb columns are in consumption order.
        idxcol_of_call = []
        _ic = 0
        for (g, cells, nch, c0) in call_list:
            idxcol_of_call.append(_ic)
            _ic += (nch * P) // 16

        def edge_phase(g_full_ap, stage_sb, post_block):
            s2_tiles = {}

            def get_s2(ch):
                b0 = (ch // C_BATCH) * C_BATCH
                if b0 not in s2_tiles:
                    cb = min(C_BATCH, nchunk - b0)
                    s2 = sbs.tile([P, C_BATCH * P], dt.bfloat16, tag="s2")
                    nc.vector.tensor_tensor(
                        out=s2[:].rearrange("p (c j) -> p c j", c=C_BATCH)[:, :cb, :],
                        in0=iota_sb[:, None, :].to_broadcast((P, cb, P)),
                        in1=dst_sb[:, b0:b0 + cb, None].to_broadcast((P, cb, P)),
                        op=mybir.AluOpType.is_equal)
                    s2_tiles.clear()
                    s2_tiles[b0] = s2
                return s2_tiles[b0], ch - b0

            ch_cons = 0
            for bgi in range(NBG):
                calls = call_list[bgi * NGRP:(bgi + 1) * NGRP]
                gbs = []
                cell_offs = []
                for gi, (g, cells, nch, c0) in enumerate(calls):
                    ci = bgi * NGRP + gi
                    n = nch * P
                    gb = sbg.tile([P, NCMAX * F1], dt.bfloat16, tag="gbuf",
                                  name="gbuf")
                    seg, par = g >> 1, g & 1
                    dma_gather(
                        nc.gpsimd,
                        gb[:, :nch * F1].rearrange("p (c f) -> p c f", f=F1),
                        g_full_ap[seg * SEGP:(seg + 1) * SEGP,
                                  par * F1:(par + 1) * F1],
                        idx_sb[:, idxcol_of_call[ci]:idxcol_of_call[ci] + n // 16],
                        num_idxs=n, elem_size=F1, elem_step=P, q=ci % 4)
                    gbs.append(gb)
                    offs = {}
                    coff = 0
                    for (b, k) in cells:
                        offs[b] = coff
                        coff += k
                    cell_offs.append(offs)
                for b in range(bgi * G_BLK, (bgi + 1) * G_BLK):
                    agg = ps_agg.tile([P, F1], dt.float32, tag="agg",
                                      name="aggps")
                    # self-loop contribution: agg = I.T @ table_rows(block)
                    nc.tensor.matmul(
                        agg[:], lhsT=ident_sb[:],
                        rhs=stage_sb[:, b * F1:(b + 1) * F1],
                        start=True, stop=False)
                    for gi, (g, cells, nch, c0) in enumerate(calls):
                        k = dict(cells)[b]
                        base = cell_offs[gi][b]
                        for c in range(k):
                            s2, coff = get_s2(ch_cons)
                            last = (gi == NGRP - 1 and c == k - 1)
                            nc.tensor.matmul(
                                agg[:], lhsT=s2[:, coff * P:(coff + 1) * P],
                                rhs=gbs[gi][:, (base + c) * F1:(base + c + 1) * F1],
                                start=False, stop=last)
                            ch_cons += 1
                    post_block(b, agg[:])

        # ---- L1 post: table2 = dinv^2 * relu(agg) (+b1 generic path) ----
        def post1(b, agg_sl):
            if with_b1:
                dv = dinv_sb[:, b:b + 1]
                t = sbb.tile([P, F1], dt.float32, tag="pb1")
                nc.vector.tensor_scalar(t[:], agg_sl, dv, None,
                                        mybir.AluOpType.mult)
                nc.vector.tensor_tensor(t[:], t[:], b1_sb[:], mybir.AluOpType.add)
                h1 = sbb.tile([P, F1], dt.bfloat16, tag="ph1")
                nc.scalar.activation(h1[:], t[:], mybir.ActivationFunctionType.Relu)
                nc.vector.tensor_scalar(g2_stage[:, b * F1:(b + 1) * F1], h1[:],
                                        dv, None, mybir.AluOpType.mult)
            else:
                nc.scalar.activation(g2_stage[:, b * F1:(b + 1) * F1], agg_sl,
                                     mybir.ActivationFunctionType.Relu,
                                     bias=0.0, scale=dinv2_sb[:, b:b + 1])

        edge_phase(g1_full, g1_stage[:], post1)
        packed_write(g2_shard, g2_stage[:])
        nc.gpsimd.collective_compute(
            "AllGather", mybir.AluOpType.bypass,
            replica_groups=[list(range(NCORES))],
            ins=[g2_shard], outs=[g2_full])

        # ---- L2 post: out = (dinv*agg2) @ W2 [+ b2] ----
        def post2(b, agg_sl):
            a2 = sbb.tile([P, F1], dt.bfloat16, tag="pa2")
            nc.scalar.activation(a2[:], agg_sl,
                                 mybir.ActivationFunctionType.Copy,
                                 bias=0.0, scale=dinv_sb[:, b:b + 1])
            a2t_ps = ps_t.tile([F1, P], dt.bfloat16, tag="tps")
            nc.tensor.transpose(a2t_ps[:], a2[:], ident_sb[:])
            a2t = sbb.tile([F1, P], dt.bfloat16, tag="pa2t")
            nc.scalar.copy(a2t[:], a2t_ps[:])
            o_ps = ps_s.tile([P, F1], dt.float32, tag="smallps")
            nc.tensor.matmul(o_ps[:, :F2], lhsT=a2t[:], rhs=w2_sb[:],
                             start=True, stop=True)
            osl = out_stage[:, b * F2:(b + 1) * F2]
            if with_b2:
                nc.vector.tensor_tensor(osl, o_ps[:, :F2], b2_sb[:],
                                        mybir.AluOpType.add)
            else:
                nc.scalar.copy(osl, o_ps[:, :F2])

        edge_phase(g2_full, g2_stage[:], post2)
        nc.sync.dma_start(
            out.rearrange("(b p) f -> p b f", p=P),
            out_stage[:].rearrange("p (b f) -> p b f", f=F2))

    nc.compile()
    globals()['_last_nc'] = nc
    return _make_runner(nc, NCORES)


_CACHE = {}


def kernel(x, edge_index, W1, b1, W2, b2):
    x = np.asarray(x, np.float32)
    edge_index = np.asarray(edge_index)
    W1 = np.asarray(W1, np.float32)
    b1 = np.asarray(b1, np.float32)
    W2 = np.asarray(W2, np.float32)
    b2 = np.asarray(b2, np.float32)

    src = edge_index[0].astype(np.int64)
    dst = edge_index[1].astype(np.int64)
    # degree includes self-loops (GCNConv adds one per node)
    deg = (np.bincount(dst, minlength=N_REAL) + 1).astype(np.float64)
    dinv = (1.0 / np.sqrt(deg)).astype(np.float32)

    perm0 = _balanced_perm(np.bincount(dst, minlength=N_REAL))  # orig -> new
    # slot twist within each block: j -> (j%2)*64 + j//2 (pair packing)
    j = perm0 % P
    perm = (perm0 // P) * P + (j % 2) * HALF + (j // 2)
    new_src = perm[src]
    new_dst = perm[dst]

    dinv_new = np.zeros(N_PAD, np.float32)
    dinv_new[perm[:N_REAL]] = dinv

    call_list, idx_arr, dst_arr, nchunk, nidx = _build_schedule(new_src, new_dst)

    with_b1 = bool(np.any(b1))
    with_b2 = bool(np.any(b2))

    ckey = (nchunk, nidx, with_b1, with_b2,
            tuple((g, tuple(cells)) for (g, cells, _, _) in call_list))
    if ckey not in _CACHE:
        _CACHE[ckey] = _compile_and_make_runner(
            call_list, nchunk, nidx, with_b1, with_b2)
    run = _CACHE[ckey]
    globals()['_last_runner'] = run

    # host-side inputs per core
    x_new = np.zeros((N_PAD, IN_C), np.float32)
    x_new[perm[:N_REAL]] = x
    xs = dinv_new[:, None] * x_new            # fold dinv into x
    iota_np = np.broadcast_to(np.arange(P, dtype=np.float32), (P, P)).copy()
    ident_np = np.eye(P, dtype=np.float32)
    w1_bf = _bf16(W1)
    w2_bf = _bf16(W2)
    iota_bf = _bf16(iota_np)
    ident_bf = _bf16(ident_np)

    in_maps = []
    for c in range(NCORES):
        lo, hi = c * NSHARD, (c + 1) * NSHARD
        dv = dinv_new[lo:hi].reshape(NBLOCKS, P).T.copy()
        m = {
            "xT": _bf16(xs[lo:hi].T.copy()),
            "w1": w1_bf,
            "w2": w2_bf,
            "dinvb": dv,
            "dinv2b": dv * dv,
            "idx16": _wrap_idx(idx_arr[c], call_list),
            "dstloc": _bf16(dst_arr[c].astype(np.float32).T.copy()),
            "iota": iota_bf,
            "ident": ident_bf,
        }
        if with_b1:
            m["b1bc"] = np.broadcast_to(b1, (P, F1)).copy()
        if with_b2:
            m["b2bc"] = np.broadcast_to(b2, (P, F2)).copy()
        in_maps.append(m)

    globals()['_last_in_maps'] = in_maps
    results, _times = run(in_maps, n_iters=0)
    out_new = np.concatenate([results[c]["out"] for c in range(NCORES)], axis=0)
    return out_new[perm[:N_REAL]].astype(np.float32)


# revision 11
# speedup vs baseline: 4.3598x; 1.4649x over previous
"""2-layer GCN (PyG GCNConv semantics) on 8 Trainium2 NeuronCores.

Strategy (self-contained; shapes hardcoded for the nn_GCNEncoder problem):
  - Nodes are relabeled (degree-balanced) and partitioned across 8 cores
    (12544 padded nodes each, 98 blocks of 128). Within a block, slot
    p = (j%2)*64 + j//2 so even/odd "pair" halves occupy partitions 0-63 /
    64-127, letting message tables pack two 64-wide node rows per 256-byte
    DRAM row (halves AllGather volume; gather rows stay 256B-stride).
  - Layer math refactored so every edge message is a 64-wide row gather:
      L1: table1 = dinv * (x @ W1)         (per-shard matmul + AllGather)
          table2 = dinv^2 * relu(segsum(table1[src]))     (b1 == 0 fast path)
      L2: out    = (dinv * segsum(table2[src])) @ W2 + b2
  - Edge aggregation per 128-node dst block: SWDGE gathers merged per
    (14-block group x (segment,parity)) to amortize the ~1us per-instruction
    GpSimd descriptor-generation cost; one-hot dst-selection matrices built
    on the DVE (is_equal vs iota); scatter-add via PSUM-accumulated TensorE
    matmuls; per-block epilogues on the Activation engine.
"""
import math
import numpy as np
from contextlib import ExitStack

N_REAL = 100000
N_PAD = 100352            # 8 * 98 * 128
NCORES = 8
NSHARD = N_PAD // NCORES  # 12544
NBLOCKS = NSHARD // 128   # 98
P = 128
HALF = 64
NPAIR_SH = NSHARD // 2    # 6272 packed pair-rows per core
NPAIR = N_PAD // 2        # 50176
SEGP = NPAIR // 2         # 25088 pair rows per int16 segment (2 segments)
NGRP = 4                  # (segment, parity) source groups
F1 = 64                   # hidden width (W1 out)
F2 = 32                   # output width (W2 out)
IN_C = 128
import os as _os
G_BLK = int(_os.environ.get("KGBLK", "7"))   # dst blocks per merged gather call
BGS = [(s, min(s + G_BLK, NBLOCKS)) for s in range(0, NBLOCKS, G_BLK)]
NBG = len(BGS)
C_BATCH = 32              # chunks per DVE one-hot build
SENTINEL = 500.0


def _bf16(a):
    import jax.numpy as jnp
    return np.asarray(jnp.asarray(a, dtype=jnp.bfloat16))


def _balanced_perm(deg):
    """Assign nodes to 784 blocks of 128 balancing per-block degree sums.
    Returns perm: orig_id -> new_id (new_id = block*128 + slot)."""
    import heapq
    nblocks_g = (N_PAD // P)  # 784
    order = np.argsort(-deg, kind="stable")
    blocks = np.empty(N_REAL, np.int64)
    heap = [(0, b) for b in range(nblocks_g)]
    heapq.heapify(heap)
    fill = np.zeros(nblocks_g, np.int64)
    deg_sorted = deg[order]
    for i in range(N_REAL):
        load, b = heapq.heappop(heap)
        blocks[order[i]] = b
        fill[b] += 1
        if fill[b] < P:
            heapq.heappush(heap, (load + int(deg_sorted[i]), b))
    o2 = np.argsort(blocks, kind="stable")
    counts = np.bincount(blocks, minlength=nblocks_g)
    starts = np.concatenate([[0], np.cumsum(counts)[:-1]])
    slots = np.arange(N_REAL) - starts[blocks[o2]]
    new_ids = blocks[o2] * P + slots
    perm_real = np.empty(N_REAL, np.int64)
    perm_real[o2] = new_ids
    used = np.zeros(N_PAD, bool)
    used[perm_real] = True
    free_ids = np.flatnonzero(~used)
    perm = np.concatenate([perm_real, free_ids])
    return perm  # length N_PAD; first N_REAL entries map real nodes


def _build_schedule(new_src, new_dst):
    """new_src/new_dst: int64 arrays over all edges (new ids, slot-twisted).
    Returns (call_list, idx arrays per core, dst arrays per core, nchunk, nidx).
    call_list: [(g, [(b, k), ...], n_chunks, chunk0)] in schedule order
    (uniform across cores); one dma_gather per entry."""
    p_s = new_src % P
    two = p_s // HALF
    q = p_s % HALF
    pr = (new_src // NSHARD) * NPAIR_SH + ((new_src % NSHARD) // P) * HALF + q
    grp = (pr // SEGP) * 2 + two          # 0..3
    idxv = pr % SEGP                      # < 25088, int16-safe

    core = new_dst // NSHARD
    block = (new_dst % NSHARD) // P       # 0..97
    dslot = new_dst % P

    key = (core * NBLOCKS + block) * NGRP + grp
    counts = np.bincount(key, minlength=NCORES * NBLOCKS * NGRP).reshape(
        NCORES, NBLOCKS, NGRP)
    k_bs = np.maximum(1, np.ceil(counts.max(axis=0) / P).astype(np.int64))

    cell_off = np.zeros((NBLOCKS, NGRP), np.int64)
    call_list = []
    running = 0
    chunk_run = 0
    for bgi in range(NBG):
        for g in range(NGRP):
            cells = []
            nch = 0
            for b in range(BGS[bgi][0], BGS[bgi][1]):
                cell_off[b, g] = running + nch * P
                cells.append((b, int(k_bs[b, g])))
                nch += int(k_bs[b, g])
            call_list.append((g, cells, nch, chunk_run))
            running += nch * P
            chunk_run += nch
    nidx = running
    nchunk = chunk_run

    eorder = np.lexsort((grp, block, core))
    s_idx = idxv[eorder]
    s_dslot = dslot[eorder]
    s_key = key[eorder]
    cnt_flat = counts.reshape(-1)
    starts = np.concatenate([[0], np.cumsum(cnt_flat)[:-1]])
    within = np.arange(len(s_idx)) - starts[s_key]
    cell_core = s_key // (NBLOCKS * NGRP)
    cbg = s_key % (NBLOCKS * NGRP)
    flat_pos = cell_off[cbg // NGRP, cbg % NGRP] + within

    # pad slots gather row 0 of their segment (harmless; dst sentinel zeroes)
    idx_arr = np.zeros((NCORES, nidx), np.int32)
    dst_arr = np.full((NCORES, nidx), int(SENTINEL), np.int32)
    idx_arr[cell_core, flat_pos] = s_idx
    dst_arr[cell_core, flat_pos] = s_dslot

    # consumption order is block-major within each block group (a block's
    # PSUM accumulation chains across the group's 4 gather calls); dst
    # columns are permuted host-side to that order.
    cons_order = []
    for bgi in range(NBG):
        calls = call_list[bgi * NGRP:(bgi + 1) * NGRP]
        for b in range(BGS[bgi][0], BGS[bgi][1]):
            for (g, cells, nch, c0) in calls:
                coff = 0
                for (b2, k) in cells:
                    if b2 == b:
                        cons_order.extend(range(c0 + coff, c0 + coff + k))
                        break
                    coff += k
    cons_order = np.asarray(cons_order)
    assert len(cons_order) == nchunk
    dst_cons = dst_arr.reshape(NCORES, nchunk, P)[:, cons_order, :]
    return call_list, idx_arr, dst_cons, nchunk, nidx


def _wrap_idx(idx_flat, call_list):
    """Per-call wrapped int16 layout: each call's n idxs -> [16, n/16] block,
    replicated to 128 partitions; concatenated along columns."""
    nidx = idx_flat.shape[0]
    cols = np.empty((16, nidx // 16), np.int16)
    off = 0
    coloff = 0
    for (g, cells, nch, c0) in call_list:
        n = nch * P
        blockv = idx_flat[off:off + n].astype(np.int16).reshape(n // 16, 16).T
        cols[:, coloff:coloff + n // 16] = blockv
        off += n
        coloff += n // 16
    return np.tile(cols, (8, 1))  # [128, nidx//16]


def _make_runner(nc, n_cores):
    """Build the bass2jax PJRT executable once; return a callable that runs it
    (optionally repeatedly, for timing)."""
    import time
    import jax
    from jax.sharding import Mesh, PartitionSpec, NamedSharding
    from jax.experimental.shard_map import shard_map
    import concourse.mybir as mybir
    from concourse.bass2jax import (_bass_exec_p, install_neuronx_cc_hook,
                                    partition_id_tensor)

    install_neuronx_cc_hook()
    partition_name = nc.partition_id_tensor.name if nc.partition_id_tensor else None

    in_names, out_names, out_avals, zero_outs = [], [], [], []
    for alloc in nc.m.functions[0].allocations:
        if not isinstance(alloc, mybir.MemoryLocationSet):
            continue
        name = alloc.memorylocations[0].name
        if alloc.kind == "ExternalInput":
            if name != partition_name:
                in_names.append(name)
        elif alloc.kind == "ExternalOutput":
            out_names.append(name)
            shape = tuple(alloc.tensor_shape)
            dtype = mybir.dt.np(alloc.dtype)
            out_avals.append(jax.core.ShapedArray(shape, dtype))
            zero_outs.append(np.zeros(shape, dtype))
    n_params = len(in_names)
    n_outs = len(out_avals)
    all_in_names = list(in_names) + list(out_names)
    if partition_name is not None:
        all_in_names.append(partition_name)

    def _body(*args):
        operands = list(args)
        if partition_name is not None:
            operands.append(partition_id_tensor())
        outs = _bass_exec_p.bind(
            *operands,
            out_avals=tuple(out_avals),
            in_names=tuple(all_in_names),
            out_names=tuple(out_names),
            lowering_input_output_aliases=(),
            sim_require_finite=False,
            sim_require_nnan=False,
            nc=nc,
        )
        return tuple(outs)

    devices = jax.devices()[:n_cores]
    mesh = Mesh(np.asarray(devices), ("core",))
    in_specs = (PartitionSpec("core"),) * (n_params + n_outs)
    out_specs = (PartitionSpec("core"),) * len(out_names)
    sharded = jax.jit(
        shard_map(_body, mesh=mesh, in_specs=in_specs, out_specs=out_specs,
                  check_rep=False),
        keep_unused=True,
    )

    def run(in_maps, n_iters=0):
        shard = NamedSharding(mesh, PartitionSpec("core"))
        per_core = [[np.asarray(m[name]) for name in in_names] for m in in_maps]
        concat_in = [
            np.concatenate([per_core[c][i] for c in range(n_cores)], axis=0)
            for i in range(n_params)
        ]
        concat_zeros = [
            np.zeros((n_cores * z.shape[0], *z.shape[1:]), z.dtype) for z in zero_outs
        ]
        args = [jax.device_put(a, shard) for a in concat_in + concat_zeros]
        out = sharded(*args)
        jax.block_until_ready(out)
        times = []
        for _ in range(n_iters):
            t0 = time.perf_counter()
            out = sharded(*args)
            jax.block_until_ready(out)
            times.append(time.perf_counter() - t0)
        results = [
            {name: np.asarray(out[i]).reshape(n_cores, *out_avals[i].shape)[c]
             for i, name in enumerate(out_names)}
            for c in range(n_cores)
        ]
        return results, times

    return run


def _compile_and_make_runner(call_list, nchunk, nidx, with_b1, with_b2):
    import concourse.bass as bass
    import concourse.bacc as bacc
    import concourse.mybir as mybir
    import concourse.tile as tile
    from concourse.bass import exact_div

    dt = mybir.dt

    def dma_gather(gp, out_ap, in_ap, idxs_ap, num_idxs, elem_size, elem_step, q):
        stride_bytes_256 = exact_div(elem_step * dt.size(in_ap.dtype), 256)
        _in_ap = gp.lower_ap_dma(in_ap, for_custom_bir_dma=True)
        return gp.add_instruction(
            mybir.InstDMAGatherAnt(
                name=gp.bass.get_next_instruction_name(),
                ins=[*_in_ap, gp.lower_ap(idxs_ap),
                     gp.lower_val_access(gp.to_reg(num_idxs))],
                outs=[gp.lower_ap(out_ap)],
                transpose=False, num_idxs=num_idxs, elem_size=elem_size,
                stride_bytes_256=stride_bytes_256, gen_mode=0,
                single_packet=False, queue_num=q))

    nc = bacc.Bacc("TRN2", target_bir_lowering=False, debug=False,
                   num_devices=NCORES, num_swdge_queues=4)

    xT = nc.dram_tensor("xT", [P, NSHARD], dt.bfloat16, kind="ExternalInput").ap()
    w1 = nc.dram_tensor("w1", [P, F1], dt.bfloat16, kind="ExternalInput").ap()
    w2 = nc.dram_tensor("w2", [F1, F2], dt.bfloat16, kind="ExternalInput").ap()
    dinvb = nc.dram_tensor("dinvb", [P, NBLOCKS], dt.float32, kind="ExternalInput").ap()
    dinv2b = nc.dram_tensor("dinv2b", [P, NBLOCKS], dt.float32, kind="ExternalInput").ap()
    idx16 = nc.dram_tensor("idx16", [P, nidx // 16], dt.int16, kind="ExternalInput").ap()
    dstloc = nc.dram_tensor("dstloc", [P, nchunk], dt.bfloat16, kind="ExternalInput").ap()
    iota = nc.dram_tensor("iota", [P, P], dt.bfloat16, kind="ExternalInput").ap()
    ident = nc.dram_tensor("ident", [P, P], dt.bfloat16, kind="ExternalInput").ap()
    if with_b1:
        b1bc = nc.dram_tensor("b1bc", [P, F1], dt.float32, kind="ExternalInput").ap()
    if with_b2:
        b2bc = nc.dram_tensor("b2bc", [P, F2], dt.float32, kind="ExternalInput").ap()
    out = nc.dram_tensor("out", [NSHARD, F2], dt.float32, kind="ExternalOutput").ap()

    g1_shard = nc.dram_tensor("g1_shard", [NPAIR_SH, P], dt.bfloat16).ap()
    g1_full = nc.dram_tensor("g1_full", [NPAIR, P], dt.bfloat16, addr_space="Shared").ap()
    g2_shard = nc.dram_tensor("g2_shard", [NPAIR_SH, P], dt.bfloat16).ap()
    g2_full = nc.dram_tensor("g2_full", [NPAIR, P], dt.bfloat16, addr_space="Shared").ap()

    NCMAX = max(nch for (_, _, nch, _) in call_list)

    with tile.TileContext(nc) as tc, ExitStack() as ctx:
        sb = ctx.enter_context(tc.tile_pool(name="sb", bufs=1))
        sbx = ctx.enter_context(tc.tile_pool(name="sbx", bufs=4))
        sbg = ctx.enter_context(tc.tile_pool(name="sbg", bufs=2 * NGRP))
        sbs = ctx.enter_context(tc.tile_pool(name="sbs", bufs=3))
        sbb = ctx.enter_context(tc.tile_pool(name="sbb", bufs=8))
        ps_agg = ctx.enter_context(tc.tile_pool(name="psagg", bufs=4,
                                                space="PSUM"))
        ps_t = ctx.enter_context(tc.tile_pool(name="pst", bufs=2, space="PSUM"))
        ps_s = ctx.enter_context(tc.tile_pool(name="pss", bufs=2, space="PSUM"))

        # ---- resident tiles ----
        w1_sb = sb.tile([P, F1], dt.bfloat16)
        w2_sb = sb.tile([F1, F2], dt.bfloat16)
        dinv_sb = sb.tile([P, NBLOCKS], dt.float32)
        dinv2_sb = sb.tile([P, NBLOCKS], dt.float32)
        idx_sb = sb.tile([P, nidx // 16], dt.int16)
        dst_sb = sb.tile([P, nchunk], dt.bfloat16)
        iota_sb = sb.tile([P, P], dt.bfloat16)
        ident_sb = sb.tile([P, P], dt.bfloat16)
        nc.sync.dma_start(w1_sb[:], w1)
        nc.sync.dma_start(w2_sb[:], w2)
        nc.sync.dma_start(dinv_sb[:], dinvb)
        nc.sync.dma_start(dinv2_sb[:], dinv2b)
        nc.sync.dma_start(idx_sb[:], idx16)
        nc.sync.dma_start(dst_sb[:], dstloc)
        nc.sync.dma_start(iota_sb[:], iota)
        nc.sync.dma_start(ident_sb[:], ident)
        if with_b1:
            b1_sb = sb.tile([P, F1], dt.float32)
            nc.sync.dma_start(b1_sb[:], b1bc)
        if with_b2:
            b2_sb = sb.tile([P, F2], dt.float32)
            nc.sync.dma_start(b2_sb[:], b2bc)

        g1_stage = sb.tile([P, NBLOCKS * F1], dt.bfloat16)
        g2_stage = sb.tile([P, NBLOCKS * F1], dt.bfloat16)
        out_stage = sb.tile([P, NBLOCKS * F2], dt.float32)

        # ---- phase 1: table1 = dinv * (x @ W1) (dinv folded into xT on host;
        # xT streamed per block to keep SBUF for gather buffers) ----
        for t in range(NBLOCKS):
            xt_t = sbx.tile([P, P], dt.bfloat16, tag="xt")
            nc.sync.dma_start(xt_t[:], xT[:, t * P:(t + 1) * P])
            h_ps = ps_s.tile([P, F1], dt.float32, tag="smallps")
            nc.tensor.matmul(h_ps[:], lhsT=xt_t[:], rhs=w1_sb[:],
                             start=True, stop=True)
            nc.scalar.copy(g1_stage[:, t * F1:(t + 1) * F1], h_ps[:])

        def packed_write(g_shard_ap, stage_sb):
            # slot p = two*64 + q holds node (b, q*2+two); pair row b*64+q
            gv = g_shard_ap.rearrange("(b q) (two f) -> q two b f", q=HALF, two=2)
            nc.sync.dma_start(
                gv[:, 0],
                stage_sb[0:HALF, :].rearrange("q (b f) -> q b f", f=F1))
            nc.sync.dma_start(
                gv[:, 1],
                stage_sb[HALF:P, :].rearrange("q (b f) -> q b f", f=F1))

        packed_write(g1_shard, g1_stage[:])
        nc.gpsimd.collective_compute(
            "AllGather", mybir.AluOpType.bypass,
            replica_groups=[list(range(NCORES))],
            ins=[g1_shard], outs=[g1_full])

        # ---- edge phase helper ----
        # gathers merged per (block-group, source-group); consumption is
        # block-major: each block's PSUM accumulation chains across the
        # group's 4 gather buffers. dst_sb columns are in consumption order.
        idxcol_of_call = []
        _ic = 0
        for (g, cells, nch, c0) in call_list:
            idxcol_of_call.append(_ic)
            _ic += (nch * P) // 16

        def edge_phase(g_full_ap, stage_sb, post_block):
            s2_tiles = {}

            def get_s2(ch):
                b0 = (ch // C_BATCH) * C_BATCH
                if b0 not in s2_tiles:
                    cb = min(C_BATCH, nchunk - b0)
                    s2 = sbs.tile([P, C_BATCH * P], dt.bfloat16, tag="s2")
                    nc.vector.tensor_tensor(
                        out=s2[:].rearrange("p (c j) -> p c j", c=C_BATCH)[:, :cb, :],
                        in0=iota_sb[:, None, :].to_broadcast((P, cb, P)),
                        in1=dst_sb[:, b0:b0 + cb, None].to_broadcast((P, cb, P)),
                        op=mybir.AluOpType.is_equal)
                    s2_tiles.clear()
                    s2_tiles[b0] = s2
                return s2_tiles[b0], ch - b0

            ch_cons = 0
            for bgi in range(NBG):
                calls = call_list[bgi * NGRP:(bgi + 1) * NGRP]
                gbs = []
                cell_offs = []
                for gi, (g, cells, nch, c0) in enumerate(calls):
                    ci = bgi * NGRP + gi
                    n = nch * P
                    gb = sbg.tile([P, NCMAX * F1], dt.bfloat16, tag="gbuf",
                                  name="gbuf")
                    seg, par = g >> 1, g & 1
                    dma_gather(
                        nc.gpsimd,
                        gb[:, :nch * F1].rearrange("p (c f) -> p c f", f=F1),
                        g_full_ap[seg * SEGP:(seg + 1) * SEGP,
                                  par * F1:(par + 1) * F1],
                        idx_sb[:, idxcol_of_call[ci]:idxcol_of_call[ci] + n // 16],
                        num_idxs=n, elem_size=F1, elem_step=P, q=ci % 4)
                    gbs.append(gb)
                    offs = {}
                    coff = 0
                    for (b, k) in cells:
                        offs[b] = coff
                        coff += k
                    cell_offs.append(offs)
                for b in range(BGS[bgi][0], BGS[bgi][1]):
                    agg = ps_agg.tile([P, F1], dt.float32, tag="agg",
                                      name="aggps")
                    # self-loop contribution: agg = I.T @ table_rows(block)
                    nc.tensor.matmul(
                        agg[:], lhsT=ident_sb[:],
                        rhs=stage_sb[:, b * F1:(b + 1) * F1],
                        start=True, stop=False)
                    for gi, (g, cells, nch, c0) in enumerate(calls):
                        k = dict(cells)[b]
                        base = cell_offs[gi][b]
                        for c in range(k):
                            s2, coff = get_s2(ch_cons)
                            last = (gi == NGRP - 1 and c == k - 1)
                            nc.tensor.matmul(
                                agg[:], lhsT=s2[:, coff * P:(coff + 1) * P],
                                rhs=gbs[gi][:, (base + c) * F1:(base + c + 1) * F1],
                                start=False, stop=last)
                            ch_cons += 1
                    post_block(b, agg[:])

        # ---- L1 post: table2 = dinv^2 * relu(agg) (+b1 generic path) ----
        def post1(b, agg_sl):
            if with_b1:
                dv = dinv_sb[:, b:b + 1]
                t = sbb.tile([P, F1], dt.float32, tag="pb1")
                nc.vector.tensor_scalar(t[:], agg_sl, dv, None,
                                        mybir.AluOpType.mult)
                nc.vector.tensor_tensor(t[:], t[:], b1_sb[:], mybir.AluOpType.add)
                h1 = sbb.tile([P, F1], dt.bfloat16, tag="ph1")
                nc.scalar.activation(h1[:], t[:], mybir.ActivationFunctionType.Relu)
                nc.vector.tensor_scalar(g2_stage[:, b * F1:(b + 1) * F1], h1[:],
                                        dv, None, mybir.AluOpType.mult)
            else:
                nc.scalar.activation(g2_stage[:, b * F1:(b + 1) * F1], agg_sl,
                                     mybir.ActivationFunctionType.Relu,
                                     bias=0.0, scale=dinv2_sb[:, b:b + 1])

        edge_phase(g1_full, g1_stage[:], post1)
        packed_write(g2_shard, g2_stage[:])
        nc.gpsimd.collective_compute(
            "AllGather", mybir.AluOpType.bypass,
            replica_groups=[list(range(NCORES))],
            ins=[g2_shard], outs=[g2_full])

        # ---- L2 post: out = (dinv*agg2) @ W2 [+ b2] ----
        def post2(b, agg_sl):
            a2 = sbb.tile([P, F1], dt.bfloat16, tag="pa2")
            nc.scalar.activation(a2[:], agg_sl,
                                 mybir.ActivationFunctionType.Copy,
                                 bias=0.0, scale=dinv_sb[:, b:b + 1])
            a2t_ps = ps_t.tile([F1, P], dt.bfloat16, tag="tps")
            nc.tensor.transpose(a2t_ps[:], a2[:], ident_sb[:])
            a2t = sbb.tile([F1, P], dt.bfloat16, tag="pa2t")
            nc.scalar.copy(a2t[:], a2t_ps[:])
            o_ps = ps_s.tile([P, F1], dt.float32, tag="smallps")
            nc.tensor.matmul(o_ps[:, :F2], lhsT=a2t[:], rhs=w2_sb[:],
                             start=True, stop=True)
            osl = out_stage[:, b * F2:(b + 1) * F2]
            if with_b2:
                nc.vector.tensor_tensor(osl, o_ps[:, :F2], b2_sb[:],
                                        mybir.AluOpType.add)
            else:
                nc.scalar.copy(osl, o_ps[:, :F2])

        edge_phase(g2_full, g2_stage[:], post2)
        nc.sync.dma_start(
            out.rearrange("(b p) f -> p b f", p=P),
            out_stage[:].rearrange("p (b f) -> p b f", f=F2))

    nc.compile()
    globals()['_last_nc'] = nc
    return _make_runner(nc, NCORES)


_CACHE = {}


def kernel(x, edge_index, W1, b1, W2, b2):
    x = np.asarray(x, np.float32)
    edge_index = np.asarray(edge_index)
    W1 = np.asarray(W1, np.float32)
    b1 = np.asarray(b1, np.float32)
    W2 = np.asarray(W2, np.float32)
    b2 = np.asarray(b2, np.float32)

    src = edge_index[0].astype(np.int64)
    dst = edge_index[1].astype(np.int64)
    # degree includes self-loops (GCNConv adds one per node)
    deg = (np.bincount(dst, minlength=N_REAL) + 1).astype(np.float64)
    dinv = (1.0 / np.sqrt(deg)).astype(np.float32)

    perm0 = _balanced_perm(np.bincount(dst, minlength=N_REAL))  # orig -> new
    # slot twist within each block: j -> (j%2)*64 + j//2 (pair packing)
    j = perm0 % P
    perm = (perm0 // P) * P + (j % 2) * HALF + (j // 2)
    new_src = perm[src]
    new_dst = perm[dst]

    dinv_new = np.zeros(N_PAD, np.float32)
    dinv_new[perm[:N_REAL]] = dinv

    call_list, idx_arr, dst_arr, nchunk, nidx = _build_schedule(new_src, new_dst)

    with_b1 = bool(np.any(b1))
    with_b2 = bool(np.any(b2))

    ckey = (nchunk, nidx, with_b1, with_b2,
            tuple((g, tuple(cells)) for (g, cells, _, _) in call_list))
    if ckey not in _CACHE:
        _CACHE[ckey] = _compile_and_make_runner(
            call_list, nchunk, nidx, with_b1, with_b2)
    run = _CACHE[ckey]
    globals()['_last_runner'] = run

    # host-side inputs per core
    x_new = np.zeros((N_PAD, IN_C), np.float32)
    x_new[perm[:N_REAL]] = x
    xs = dinv_new[:, None] * x_new            # fold dinv into x
    iota_np = np.broadcast_to(np.arange(P, dtype=np.float32), (P, P)).copy()
    ident_np = np.eye(P, dtype=np.float32)
    w1_bf = _bf16(W1)
    w2_bf = _bf16(W2)
    iota_bf = _bf16(iota_np)
    ident_bf = _bf16(ident_np)

    in_maps = []
    for c in range(NCORES):
        lo, hi = c * NSHARD, (c + 1) * NSHARD
        dv = dinv_new[lo:hi].reshape(NBLOCKS, P).T.copy()
        m = {
            "xT": _bf16(xs[lo:hi].T.copy()),
            "w1": w1_bf,
            "w2": w2_bf,
            "dinvb": dv,
            "dinv2b": dv * dv,
            "idx16": _wrap_idx(idx_arr[c], call_list),
            "dstloc": _bf16(dst_arr[c].astype(np.float32).T.copy()),
            "iota": iota_bf,
            "ident": ident_bf,
        }
        if with_b1:
            m["b1bc"] = np.broadcast_to(b1, (P, F1)).copy()
        if with_b2:
            m["b2bc"] = np.broadcast_to(b2, (P, F2)).copy()
        in_maps.append(m)

    globals()['_last_in_maps'] = in_maps
    results, _times = run(in_maps, n_iters=0)
    out_new = np.concatenate([results[c]["out"] for c in range(NCORES)], axis=0)
    return out_new[perm[:N_REAL]].astype(np.float32)


# revision 13
# speedup vs baseline: 4.7478x; 1.0890x over previous
"""2-layer GCN (PyG GCNConv semantics) on 8 Trainium2 NeuronCores.

Strategy (self-contained; shapes hardcoded for the nn_GCNEncoder problem):
  - Nodes are relabeled (degree-balanced) and partitioned across 8 cores
    (12544 padded nodes each, 98 blocks of 128). Within a block, slot
    p = (j%2)*64 + j//2 so even/odd "pair" halves occupy partitions 0-63 /
    64-127, letting message tables pack two 64-wide node rows per 256-byte
    DRAM row (halves AllGather volume; gather rows stay 256B-stride).
  - Layer math refactored so every edge message is a 64-wide row gather:
      L1: table1 = dinv * (x @ W1)         (per-shard matmul + AllGather)
          table2 = dinv^2 * relu(segsum(table1[src]))     (b1 == 0 fast path)
      L2: out    = (dinv * segsum(table2[src])) @ W2 + b2
  - Edge aggregation per 128-node dst block: SWDGE gathers merged per
    (14-block group x (segment,parity)) to amortize the ~1us per-instruction
    GpSimd descriptor-generation cost; one-hot dst-selection matrices built
    on the DVE (is_equal vs iota); scatter-add via PSUM-accumulated TensorE
    matmuls; per-block epilogues on the Activation engine.
"""
import math
import numpy as np
from contextlib import ExitStack

N_REAL = 100000
N_PAD = 100352            # 8 * 98 * 128
NCORES = 8
NSHARD = N_PAD // NCORES  # 12544
NBLOCKS = NSHARD // 128   # 98
P = 128
HALF = 64
NPAIR_SH = NSHARD // 2    # 6272 packed pair-rows per core
NPAIR = N_PAD // 2        # 50176
SEGP = NPAIR // 2         # 25088 pair rows per int16 segment (2 segments)
NGRP = 4                  # (segment, parity) source groups
F1 = 64                   # hidden width (W1 out)
F2 = 32                   # output width (W2 out)
IN_C = 128
import os as _os
G_BLK = int(_os.environ.get("KGBLK", "7"))   # dst blocks per merged gather call
BGS = [(s, min(s + G_BLK, NBLOCKS)) for s in range(0, NBLOCKS, G_BLK)]
NBG = len(BGS)
C_BATCH = 32              # chunks per DVE one-hot build
SENTINEL = 500.0


def _bf16(a):
    import ml_dtypes
    return np.asarray(a, np.float32).astype(ml_dtypes.bfloat16)


def _balanced_perm(deg):
    """Assign nodes to 784 blocks of 128 balancing per-block degree sums.
    Returns perm: orig_id -> new_id (new_id = block*128 + slot)."""
    import heapq
    nblocks_g = (N_PAD // P)  # 784
    order = np.argsort(-deg, kind="stable")
    blocks = np.empty(N_REAL, np.int64)
    heap = [(0, b) for b in range(nblocks_g)]
    heapq.heapify(heap)
    fill = np.zeros(nblocks_g, np.int64)
    deg_sorted = deg[order]
    for i in range(N_REAL):
        load, b = heapq.heappop(heap)
        blocks[order[i]] = b
        fill[b] += 1
        if fill[b] < P:
            heapq.heappush(heap, (load + int(deg_sorted[i]), b))
    o2 = np.argsort(blocks, kind="stable")
    counts = np.bincount(blocks, minlength=nblocks_g)
    starts = np.concatenate([[0], np.cumsum(counts)[:-1]])
    slots = np.arange(N_REAL) - starts[blocks[o2]]
    new_ids = blocks[o2] * P + slots
    perm_real = np.empty(N_REAL, np.int64)
    perm_real[o2] = new_ids
    used = np.zeros(N_PAD, bool)
    used[perm_real] = True
    free_ids = np.flatnonzero(~used)
    perm = np.concatenate([perm_real, free_ids])
    return perm  # length N_PAD; first N_REAL entries map real nodes


def _build_schedule(new_src, new_dst):
    """new_src/new_dst: int64 arrays over all edges (new ids, slot-twisted).
    Returns (call_list, idx arrays per core, dst arrays per core, nchunk, nidx).
    call_list: [(g, [(b, k), ...], n_chunks, chunk0)] in schedule order
    (uniform across cores); one dma_gather per entry."""
    p_s = new_src % P
    two = p_s // HALF
    q = p_s % HALF
    pr = (new_src // NSHARD) * NPAIR_SH + ((new_src % NSHARD) // P) * HALF + q
    grp = (pr // SEGP) * 2 + two          # 0..3
    idxv = pr % SEGP                      # < 25088, int16-safe

    core = new_dst // NSHARD
    block = (new_dst % NSHARD) // P       # 0..97
    dslot = new_dst % P

    key = (core * NBLOCKS + block) * NGRP + grp
    counts = np.bincount(key, minlength=NCORES * NBLOCKS * NGRP).reshape(
        NCORES, NBLOCKS, NGRP)
    k_bs = np.maximum(1, np.ceil(counts.max(axis=0) / P).astype(np.int64))

    cell_off = np.zeros((NBLOCKS, NGRP), np.int64)
    call_list = []
    running = 0
    chunk_run = 0
    for bgi in range(NBG):
        for g in range(NGRP):
            cells = []
            nch = 0
            for b in range(BGS[bgi][0], BGS[bgi][1]):
                cell_off[b, g] = running + nch * P
                cells.append((b, int(k_bs[b, g])))
                nch += int(k_bs[b, g])
            call_list.append((g, cells, nch, chunk_run))
            running += nch * P
            chunk_run += nch
    nidx = running
    nchunk = chunk_run

    eorder = np.lexsort((grp, block, core))
    s_idx = idxv[eorder]
    s_dslot = dslot[eorder]
    s_key = key[eorder]
    cnt_flat = counts.reshape(-1)
    starts = np.concatenate([[0], np.cumsum(cnt_flat)[:-1]])
    within = np.arange(len(s_idx)) - starts[s_key]
    cell_core = s_key // (NBLOCKS * NGRP)
    cbg = s_key % (NBLOCKS * NGRP)
    flat_pos = cell_off[cbg // NGRP, cbg % NGRP] + within

    # pad slots gather row 0 of their segment (harmless; dst sentinel zeroes)
    idx_arr = np.zeros((NCORES, nidx), np.int32)
    dst_arr = np.full((NCORES, nidx), int(SENTINEL), np.int32)
    idx_arr[cell_core, flat_pos] = s_idx
    dst_arr[cell_core, flat_pos] = s_dslot

    # consumption order is block-major within each block group (a block's
    # PSUM accumulation chains across the group's 4 gather calls); dst
    # columns are permuted host-side to that order.
    cons_order = []
    for bgi in range(NBG):
        calls = call_list[bgi * NGRP:(bgi + 1) * NGRP]
        for b in range(BGS[bgi][0], BGS[bgi][1]):
            for (g, cells, nch, c0) in calls:
                coff = 0
                for (b2, k) in cells:
                    if b2 == b:
                        cons_order.extend(range(c0 + coff, c0 + coff + k))
                        break
                    coff += k
    cons_order = np.asarray(cons_order)
    assert len(cons_order) == nchunk
    dst_cons = dst_arr.reshape(NCORES, nchunk, P)[:, cons_order, :]
    return call_list, idx_arr, dst_cons, nchunk, nidx


def _wrap_idx(idx_flat, call_list):
    """Per-call wrapped int16 layout: each call's n idxs -> [16, n/16] block,
    replicated to 128 partitions; concatenated along columns."""
    nidx = idx_flat.shape[0]
    cols = np.empty((16, nidx // 16), np.int16)
    off = 0
    coloff = 0
    for (g, cells, nch, c0) in call_list:
        n = nch * P
        blockv = idx_flat[off:off + n].astype(np.int16).reshape(n // 16, 16).T
        cols[:, coloff:coloff + n // 16] = blockv
        off += n
        coloff += n // 16
    return np.tile(cols, (8, 1))  # [128, nidx//16]


def _make_runner(nc, n_cores):
    """Build the bass2jax PJRT executable once; return a callable that runs it
    (optionally repeatedly, for timing)."""
    import time
    import jax
    from jax.sharding import Mesh, PartitionSpec, NamedSharding
    from jax.experimental.shard_map import shard_map
    import concourse.mybir as mybir
    from concourse.bass2jax import (_bass_exec_p, install_neuronx_cc_hook,
                                    partition_id_tensor)

    install_neuronx_cc_hook()
    partition_name = nc.partition_id_tensor.name if nc.partition_id_tensor else None

    in_names, out_names, out_avals, zero_outs = [], [], [], []
    for alloc in nc.m.functions[0].allocations:
        if not isinstance(alloc, mybir.MemoryLocationSet):
            continue
        name = alloc.memorylocations[0].name
        if alloc.kind == "ExternalInput":
            if name != partition_name:
                in_names.append(name)
        elif alloc.kind == "ExternalOutput":
            out_names.append(name)
            shape = tuple(alloc.tensor_shape)
            dtype = mybir.dt.np(alloc.dtype)
            out_avals.append(jax.core.ShapedArray(shape, dtype))
            zero_outs.append(np.zeros(shape, dtype))
    n_params = len(in_names)
    n_outs = len(out_avals)
    all_in_names = list(in_names) + list(out_names)
    if partition_name is not None:
        all_in_names.append(partition_name)

    def _body(*args):
        operands = list(args)
        if partition_name is not None:
            operands.append(partition_id_tensor())
        outs = _bass_exec_p.bind(
            *operands,
            out_avals=tuple(out_avals),
            in_names=tuple(all_in_names),
            out_names=tuple(out_names),
            lowering_input_output_aliases=(),
            sim_require_finite=False,
            sim_require_nnan=False,
            nc=nc,
        )
        return tuple(outs)

    devices = jax.devices()[:n_cores]
    mesh = Mesh(np.asarray(devices), ("core",))
    in_specs = (PartitionSpec("core"),) * (n_params + n_outs)
    out_specs = (PartitionSpec("core"),) * len(out_names)
    sharded = jax.jit(
        shard_map(_body, mesh=mesh, in_specs=in_specs, out_specs=out_specs,
                  check_rep=False),
        keep_unused=True,
    )

    def run(in_maps, n_iters=0):
        shard = NamedSharding(mesh, PartitionSpec("core"))
        per_core = [[np.asarray(m[name]) for name in in_names] for m in in_maps]
        concat_in = [
            np.concatenate([per_core[c][i] for c in range(n_cores)], axis=0)
            for i in range(n_params)
        ]
        concat_zeros = [
            np.zeros((n_cores * z.shape[0], *z.shape[1:]), z.dtype) for z in zero_outs
        ]
        args = [jax.device_put(a, shard) for a in concat_in + concat_zeros]
        out = sharded(*args)
        jax.block_until_ready(out)
        times = []
        for _ in range(n_iters):
            t0 = time.perf_counter()
            out = sharded(*args)
            jax.block_until_ready(out)
            times.append(time.perf_counter() - t0)
        results = [
            {name: np.asarray(out[i]).reshape(n_cores, *out_avals[i].shape)[c]
             for i, name in enumerate(out_names)}
            for c in range(n_cores)
        ]
        return results, times

    return run


def _compile_and_make_runner(call_list, nchunk, nidx, with_b1, with_b2):
    import concourse.bass as bass
    import concourse.bacc as bacc
    import concourse.mybir as mybir
    import concourse.tile as tile
    from concourse.bass import exact_div

    dt = mybir.dt

    def dma_gather(gp, out_ap, in_ap, idxs_ap, num_idxs, elem_size, elem_step, q):
        stride_bytes_256 = exact_div(elem_step * dt.size(in_ap.dtype), 256)
        _in_ap = gp.lower_ap_dma(in_ap, for_custom_bir_dma=True)
        return gp.add_instruction(
            mybir.InstDMAGatherAnt(
                name=gp.bass.get_next_instruction_name(),
                ins=[*_in_ap, gp.lower_ap(idxs_ap),
                     gp.lower_val_access(gp.to_reg(num_idxs))],
                outs=[gp.lower_ap(out_ap)],
                transpose=False, num_idxs=num_idxs, elem_size=elem_size,
                stride_bytes_256=stride_bytes_256, gen_mode=0,
                single_packet=False, queue_num=q))

    nc = bacc.Bacc("TRN2", target_bir_lowering=False, debug=False,
                   num_devices=NCORES, num_swdge_queues=4)

    xT = nc.dram_tensor("xT", [P, NSHARD], dt.bfloat16, kind="ExternalInput").ap()
    w1 = nc.dram_tensor("w1", [P, F1], dt.bfloat16, kind="ExternalInput").ap()
    w2 = nc.dram_tensor("w2", [F1, F2], dt.bfloat16, kind="ExternalInput").ap()
    dinvb = nc.dram_tensor("dinvb", [P, NBLOCKS], dt.float32, kind="ExternalInput").ap()
    dinv2b = nc.dram_tensor("dinv2b", [P, NBLOCKS], dt.float32, kind="ExternalInput").ap()
    idx16 = nc.dram_tensor("idx16", [P, nidx // 16], dt.int16, kind="ExternalInput").ap()
    dstloc = nc.dram_tensor("dstloc", [P, nchunk], dt.bfloat16, kind="ExternalInput").ap()
    iota = nc.dram_tensor("iota", [P, P], dt.bfloat16, kind="ExternalInput").ap()
    ident = nc.dram_tensor("ident", [P, P], dt.bfloat16, kind="ExternalInput").ap()
    if with_b1:
        b1bc = nc.dram_tensor("b1bc", [P, F1], dt.float32, kind="ExternalInput").ap()
    if with_b2:
        b2bc = nc.dram_tensor("b2bc", [P, F2], dt.float32, kind="ExternalInput").ap()
    out = nc.dram_tensor("out", [NSHARD, F2], dt.float32, kind="ExternalOutput").ap()

    g1_shard = nc.dram_tensor("g1_shard", [NPAIR_SH, P], dt.bfloat16).ap()
    g1_full = nc.dram_tensor("g1_full", [NPAIR, P], dt.bfloat16, addr_space="Shared").ap()
    g2_shard = nc.dram_tensor("g2_shard", [NPAIR_SH, P], dt.bfloat16).ap()
    g2_full = nc.dram_tensor("g2_full", [NPAIR, P], dt.bfloat16, addr_space="Shared").ap()

    NCMAX = max(nch for (_, _, nch, _) in call_list)

    with tile.TileContext(nc) as tc, ExitStack() as ctx:
        sb = ctx.enter_context(tc.tile_pool(name="sb", bufs=1))
        sbx = ctx.enter_context(tc.tile_pool(name="sbx", bufs=4))
        sbg = ctx.enter_context(tc.tile_pool(name="sbg", bufs=2 * NGRP))
        sbs = ctx.enter_context(tc.tile_pool(name="sbs", bufs=5))
        sbb = ctx.enter_context(tc.tile_pool(name="sbb", bufs=8))
        ps_agg = ctx.enter_context(tc.tile_pool(name="psagg", bufs=4,
                                                space="PSUM"))
        ps_t = ctx.enter_context(tc.tile_pool(name="pst", bufs=2, space="PSUM"))
        ps_s = ctx.enter_context(tc.tile_pool(name="pss", bufs=2, space="PSUM"))

        # ---- resident tiles ----
        w1_sb = sb.tile([P, F1], dt.bfloat16)
        w2_sb = sb.tile([F1, F2], dt.bfloat16)
        dinv_sb = sb.tile([P, NBLOCKS], dt.float32)
        dinv2_sb = sb.tile([P, NBLOCKS], dt.float32)
        idx_sb = sb.tile([P, nidx // 16], dt.int16)
        dst_sb = sb.tile([P, nchunk], dt.bfloat16)
        iota_sb = sb.tile([P, P], dt.bfloat16)
        ident_sb = sb.tile([P, P], dt.bfloat16)
        nc.sync.dma_start(w1_sb[:], w1)
        nc.sync.dma_start(w2_sb[:], w2)
        nc.sync.dma_start(dinv_sb[:], dinvb)
        nc.sync.dma_start(dinv2_sb[:], dinv2b)
        nc.sync.dma_start(idx_sb[:], idx16)
        nc.sync.dma_start(dst_sb[:], dstloc)
        nc.sync.dma_start(iota_sb[:], iota)
        nc.sync.dma_start(ident_sb[:], ident)
        if with_b1:
            b1_sb = sb.tile([P, F1], dt.float32)
            nc.sync.dma_start(b1_sb[:], b1bc)
        if with_b2:
            b2_sb = sb.tile([P, F2], dt.float32)
            nc.sync.dma_start(b2_sb[:], b2bc)

        g1_stage = sb.tile([P, NBLOCKS * F1], dt.bfloat16)
        g2_stage = sb.tile([P, NBLOCKS * F1], dt.bfloat16)
        out_stage = sb.tile([P, NBLOCKS * F2], dt.float32)

        # ---- phase 1: table1 = dinv * (x @ W1) (dinv folded into xT on host;
        # xT streamed per block to keep SBUF for gather buffers) ----
        for t in range(NBLOCKS):
            xt_t = sbx.tile([P, P], dt.bfloat16, tag="xt")
            nc.sync.dma_start(xt_t[:], xT[:, t * P:(t + 1) * P])
            h_ps = ps_s.tile([P, F1], dt.float32, tag="smallps")
            nc.tensor.matmul(h_ps[:], lhsT=xt_t[:], rhs=w1_sb[:],
                             start=True, stop=True)
            nc.scalar.copy(g1_stage[:, t * F1:(t + 1) * F1], h_ps[:])

        def packed_write(g_shard_ap, stage_sb):
            # slot p = two*64 + q holds node (b, q*2+two); pair row b*64+q
            gv = g_shard_ap.rearrange("(b q) (two f) -> q two b f", q=HALF, two=2)
            nc.sync.dma_start(
                gv[:, 0],
                stage_sb[0:HALF, :].rearrange("q (b f) -> q b f", f=F1))
            nc.sync.dma_start(
                gv[:, 1],
                stage_sb[HALF:P, :].rearrange("q (b f) -> q b f", f=F1))

        packed_write(g1_shard, g1_stage[:])
        nc.gpsimd.collective_compute(
            "AllGather", mybir.AluOpType.bypass,
            replica_groups=[list(range(NCORES))],
            ins=[g1_shard], outs=[g1_full])

        # ---- edge phase helper ----
        # gathers merged per (block-group, source-group); consumption is
        # block-major: each block's PSUM accumulation chains across the
        # group's 4 gather buffers. dst_sb columns are in consumption order.
        idxcol_of_call = []
        _ic = 0
        for (g, cells, nch, c0) in call_list:
            idxcol_of_call.append(_ic)
            _ic += (nch * P) // 16

        def edge_phase(g_full_ap, stage_sb, post_block):
            s2_tiles = {}

            def get_s2(ch):
                b0 = (ch // C_BATCH) * C_BATCH
                if b0 not in s2_tiles:
                    cb = min(C_BATCH, nchunk - b0)
                    s2 = sbs.tile([P, C_BATCH * P], dt.bfloat16, tag="s2")
                    nc.vector.tensor_tensor(
                        out=s2[:].rearrange("p (c j) -> p c j", c=C_BATCH)[:, :cb, :],
                        in0=iota_sb[:, None, :].to_broadcast((P, cb, P)),
                        in1=dst_sb[:, b0:b0 + cb, None].to_broadcast((P, cb, P)),
                        op=mybir.AluOpType.is_equal)
                    s2_tiles.clear()
                    s2_tiles[b0] = s2
                return s2_tiles[b0], ch - b0

            ch_cons = 0
            for bgi in range(NBG):
                calls = call_list[bgi * NGRP:(bgi + 1) * NGRP]
                gbs = []
                cell_offs = []
                for gi, (g, cells, nch, c0) in enumerate(calls):
                    ci = bgi * NGRP + gi
                    n = nch * P
                    gb = sbg.tile([P, NCMAX * F1], dt.bfloat16, tag="gbuf",
                                  name="gbuf")
                    seg, par = g >> 1, g & 1
                    dma_gather(
                        nc.gpsimd,
                        gb[:, :nch * F1].rearrange("p (c f) -> p c f", f=F1),
                        g_full_ap[seg * SEGP:(seg + 1) * SEGP,
                                  par * F1:(par + 1) * F1],
                        idx_sb[:, idxcol_of_call[ci]:idxcol_of_call[ci] + n // 16],
                        num_idxs=n, elem_size=F1, elem_step=P, q=ci % 4)
                    gbs.append(gb)
                    offs = {}
                    coff = 0
                    for (b, k) in cells:
                        offs[b] = coff
                        coff += k
                    cell_offs.append(offs)
                for b in range(BGS[bgi][0], BGS[bgi][1]):
                    agg = ps_agg.tile([P, F1], dt.float32, tag="agg",
                                      name="aggps")
                    # self-loop contribution: agg = I.T @ table_rows(block)
                    nc.tensor.matmul(
                        agg[:], lhsT=ident_sb[:],
                        rhs=stage_sb[:, b * F1:(b + 1) * F1],
                        start=True, stop=False)
                    for gi, (g, cells, nch, c0) in enumerate(calls):
                        k = dict(cells)[b]
                        base = cell_offs[gi][b]
                        for c in range(k):
                            s2, coff = get_s2(ch_cons)
                            last = (gi == NGRP - 1 and c == k - 1)
                            nc.tensor.matmul(
                                agg[:], lhsT=s2[:, coff * P:(coff + 1) * P],
                                rhs=gbs[gi][:, (base + c) * F1:(base + c + 1) * F1],
                                start=False, stop=last)
                            ch_cons += 1
                    post_block(b, agg[:])

        # ---- L1 post: table2 = dinv^2 * relu(agg) (+b1 generic path) ----
        def post1(b, agg_sl):
            if with_b1:
                dv = dinv_sb[:, b:b + 1]
                t = sbb.tile([P, F1], dt.float32, tag="pb1")
                nc.vector.tensor_scalar(t[:], agg_sl, dv, None,
                                        mybir.AluOpType.mult)
                nc.vector.tensor_tensor(t[:], t[:], b1_sb[:], mybir.AluOpType.add)
                h1 = sbb.tile([P, F1], dt.bfloat16, tag="ph1")
                nc.scalar.activation(h1[:], t[:], mybir.ActivationFunctionType.Relu)
                nc.vector.tensor_scalar(g2_stage[:, b * F1:(b + 1) * F1], h1[:],
                                        dv, None, mybir.AluOpType.mult)
            else:
                nc.scalar.activation(g2_stage[:, b * F1:(b + 1) * F1], agg_sl,
                                     mybir.ActivationFunctionType.Relu,
                                     bias=0.0, scale=dinv2_sb[:, b:b + 1])

        edge_phase(g1_full, g1_stage[:], post1)
        packed_write(g2_shard, g2_stage[:])
        nc.gpsimd.collective_compute(
            "AllGather", mybir.AluOpType.bypass,
            replica_groups=[list(range(NCORES))],
            ins=[g2_shard], outs=[g2_full])

        # ---- L2 post: out = (dinv*agg2) @ W2 [+ b2] ----
        def post2(b, agg_sl):
            a2 = sbb.tile([P, F1], dt.bfloat16, tag="pa2")
            nc.scalar.activation(a2[:], agg_sl,
                                 mybir.ActivationFunctionType.Copy,
                                 bias=0.0, scale=dinv_sb[:, b:b + 1])
            a2t_ps = ps_t.tile([F1, P], dt.bfloat16, tag="tps")
            nc.tensor.transpose(a2t_ps[:], a2[:], ident_sb[:])
            a2t = sbb.tile([F1, P], dt.bfloat16, tag="pa2t")
            nc.scalar.copy(a2t[:], a2t_ps[:])
            o_ps = ps_s.tile([P, F1], dt.float32, tag="smallps")
            nc.tensor.matmul(o_ps[:, :F2], lhsT=a2t[:], rhs=w2_sb[:],
                             start=True, stop=True)
            osl = out_stage[:, b * F2:(b + 1) * F2]
            if with_b2:
                nc.vector.tensor_tensor(osl, o_ps[:, :F2], b2_sb[:],
                                        mybir.AluOpType.add)
            else:
                nc.scalar.copy(osl, o_ps[:, :F2])

        edge_phase(g2_full, g2_stage[:], post2)
        nc.sync.dma_start(
            out.rearrange("(b p) f -> p b f", p=P),
            out_stage[:].rearrange("p (b f) -> p b f", f=F2))

    nc.compile()
    globals()['_last_nc'] = nc
    return _make_runner(nc, NCORES)


_CACHE = {}


def kernel(x, edge_index, W1, b1, W2, b2):
    x = np.asarray(x, np.float32)
    edge_index = np.asarray(edge_index)
    W1 = np.asarray(W1, np.float32)
    b1 = np.asarray(b1, np.float32)
    W2 = np.asarray(W2, np.float32)
    b2 = np.asarray(b2, np.float32)

    src = edge_index[0].astype(np.int64)
    dst = edge_index[1].astype(np.int64)
    # degree includes self-loops (GCNConv adds one per node)
    deg = (np.bincount(dst, minlength=N_REAL) + 1).astype(np.float64)
    dinv = (1.0 / np.sqrt(deg)).astype(np.float32)

    perm0 = _balanced_perm(np.bincount(dst, minlength=N_REAL))  # orig -> new
    # slot twist within each block: j -> (j%2)*64 + j//2 (pair packing)
    j = perm0 % P
    perm = (perm0 // P) * P + (j % 2) * HALF + (j // 2)
    new_src = perm[src]
    new_dst = perm[dst]

    dinv_new = np.zeros(N_PAD, np.float32)
    dinv_new[perm[:N_REAL]] = dinv

    call_list, idx_arr, dst_arr, nchunk, nidx = _build_schedule(new_src, new_dst)

    with_b1 = bool(np.any(b1))
    with_b2 = bool(np.any(b2))

    ckey = (nchunk, nidx, with_b1, with_b2,
            tuple((g, tuple(cells)) for (g, cells, _, _) in call_list))
    if ckey not in _CACHE:
        _CACHE[ckey] = _compile_and_make_runner(
            call_list, nchunk, nidx, with_b1, with_b2)
    run = _CACHE[ckey]
    globals()['_last_runner'] = run

    # host-side inputs per core
    x_new = np.zeros((N_PAD, IN_C), np.float32)
    x_new[perm[:N_REAL]] = x
    xs = dinv_new[:, None] * x_new            # fold dinv into x
    iota_np = np.broadcast_to(np.arange(P, dtype=np.float32), (P, P)).copy()
    ident_np = np.eye(P, dtype=np.float32)
    w1_bf = _bf16(W1)
    w2_bf = _bf16(W2)
    iota_bf = _bf16(iota_np)
    ident_bf = _bf16(ident_np)

    in_maps = []
    for c in range(NCORES):
        lo, hi = c * NSHARD, (c + 1) * NSHARD
        dv = dinv_new[lo:hi].reshape(NBLOCKS, P).T.copy()
        m = {
            "xT": _bf16(xs[lo:hi].T.copy()),
            "w1": w1_bf,
            "w2": w2_bf,
            "dinvb": dv,
            "dinv2b": dv * dv,
            "idx16": _wrap_idx(idx_arr[c], call_list),
            "dstloc": _bf16(dst_arr[c].astype(np.float32).T.copy()),
            "iota": iota_bf,
            "ident": ident_bf,
        }
        if with_b1:
            m["b1bc"] = np.broadcast_to(b1, (P, F1)).copy()
        if with_b2:
            m["b2bc"] = np.broadcast_to(b2, (P, F2)).copy()
        in_maps.append(m)

    globals()['_last_in_maps'] = in_maps
    results, _times = run(in_maps, n_iters=0)
    out_new = np.concatenate([results[c]["out"] for c in range(NCORES)], axis=0)
    return out_new[perm[:N_REAL]].astype(np.float32)
